# revision 24
# baseline (speedup 1.0000x reference)
"""Trainium2 Bass kernel for the spiral-conv mesh autoencoder (nn_AE_45810121179173).

Data-parallel over batch: core i runs the full network for batch element i.
- gathers via InstDMAGatherAnt (SWDGE dma_gather) round-robined over 4 SWDGE
  queues; per-queue throughput ~8ns/idx (64B elems) so saturation needs 4
  concurrent gather instructions -> small uniform chunks + deep buffering.
- tables stored at 256B pitch ([*, 128] bf16) as dma_gather requires the
  source pitch to be a multiple of 256B; elem_size = real channel count.
- indices are int16; 40000-row tables use a rotated 65536-row buffer (row v
  stored at (v+32768)%65536, gather base view at +32768) so uint16-wrapped
  indices address the full range via signed-offset wraparound.
- first conv's x-gather is done host-side and laid out feature-major, so
  conv ce0 needs no gather and no PE transposes.
- pools: host-side degree-sorted padded-CSR gather; chunks group equal-width
  tiles so the weighted tree-sum is one DVE op per halving level per chunk.
- convs: gather (vertex-major) -> PE transpose to feature-major (PSUM f32,
  two tiles per bank-pair) -> scalar-engine copy to SBUF bf16 -> PSUM-
  accumulated matmuls -> pair-wide bias + ELU epilogue (scalar+DVE bf16)
  -> PE transpose back -> single table write per group.
Self-contained: hardcodes shapes; needs numpy + concourse runtime only.
"""
import sys

for _p in ("/opt/trn_rl_repo", "/root/.axon_site/_ro/trn_rl_repo"):
    if _p not in sys.path:
        sys.path.insert(0, _p)

import numpy as np

V = [40000, 10000, 2500, 625, 160]
L = 9
LAT = 256
B = 8
P = 128
TCONV = 512
G = 4
EP = 128          # table pitch in bf16 elements (256B)
ROT = 32768       # rotation for 40000-row tables
VB = 65536        # rotated table rows
NQ = 4            # SWDGE queues
POOL_SUB = 32     # target gather columns per pool chunk (32*128=4096 idx)
POOL_WMAX = 32    # a single tile may be up to this wide


def _pow2_up(x):
    n = 1
    while n < x:
        n *= 2
    return n


def _tpad(v, m):
    return ((v + m - 1) // m) * m


def _wrap16(flat):
    """[n] values -> [128, n//16+1] int16 FW layout; appends a positive
    sentinel column so the FW's trailing-negative-pad scan never drops real
    entries whose uint16 wrap is negative."""
    n = len(flat)
    assert n % 16 == 0
    u = np.asarray(flat).astype(np.uint16).astype(np.int16)
    u = np.concatenate([u, np.zeros(16, np.int16)])
    w = u.reshape(n // 16 + 1, 16).T  # [16, cols]; entry i at [i%16, i//16]
    return np.tile(w, (8, 1))


def _prep_pool(row, col, val, v_out):
    nnz = len(row)
    deg = np.bincount(row, minlength=v_out)
    perm = np.argsort(-deg, kind="stable").astype(np.int64)
    inv = np.empty(v_out, np.int64)
    inv[perm] = np.arange(v_out)
    order = np.argsort(row, kind="stable")
    col_s, val_s = col[order].astype(np.int64), np.asarray(val)[order].astype(np.float32)
    starts = np.zeros(v_out + 1, np.int64)
    np.cumsum(deg, out=starts[1:])
    ntiles = (v_out + P - 1) // P
    widths, idx_l, val_l = [], [], []
    for t in range(ntiles):
        dests = perm[t * P:(t + 1) * P]
        k = deg[dests]
        W = _pow2_up(max(1, int(k.max()) if len(k) else 1))
        assert W <= POOL_WMAX, f"pool tile width {W} exceeds {POOL_WMAX}"
        npd = len(dests)
        mask = np.arange(W)[None, :] < k[:, None]
        flat = np.minimum(starts[dests][:, None] + np.arange(W)[None, :], max(nnz - 1, 0))
        I = np.zeros((P, W), np.int64)
        A = np.zeros((P, W), np.float32)
        I[:npd] = np.where(mask, col_s[flat], 0)
        A[:npd] = np.where(mask, val_s[flat], 0.0)
        widths.append(W)
        idx_l.append(I)
        val_l.append(A)
    return dict(perm=perm, inv=inv, widths=widths,
                idx=np.concatenate(idx_l, 1),
                val=np.concatenate(val_l, 1).astype(np.float32), ntiles=ntiles)


def _pool_chunks(widths, brk=None):
    """Uniform-W chunking: consecutive equal-width tiles, wsum <= POOL_SUB,
    break at tile brk. A single tile wider than POOL_SUB gets its own chunk."""
    chunks = []
    t0 = 0
    ntiles = len(widths)
    while t0 < ntiles:
        W = widths[t0]
        t1, wsum = t0, 0
        while (t1 < ntiles and widths[t1] == W and wsum + W <= POOL_SUB
               and not (brk is not None and t1 == brk and t1 > t0)):
            wsum += W
            t1 += 1
        if t1 == t0:  # W > POOL_SUB
            t1, wsum = t0 + 1, W
        chunks.append((t0, t1, wsum))
        t0 = t1
    return chunks


def _pool_idx16(pp, brk=None):
    """Build per-chunk wrapped idx + chunk list; idx flat order i=col*128+p."""
    widths = pp["widths"]
    chunks = _pool_chunks(widths, brk)
    offs = np.concatenate([[0], np.cumsum(widths)]).astype(int)
    blocks = []
    for (t0, t1, wsum) in chunks:
        cols = pp["idx"][:, offs[t0]:offs[t1]]  # [128, wsum]
        blocks.append(_wrap16(cols.T.reshape(-1)))  # [128, wsum*8+1]
    return chunks, np.concatenate(blocks, 1).astype(np.int16), offs


def _conv_idx16(sp, inv):
    """[128, ntiles*289] int16: per-tile wrap blocks; i=(gr*9+j)*128+p."""
    Vq = sp.shape[0]
    spr = (inv[sp] if inv is not None else np.asarray(sp).astype(np.int64))
    ntiles = (Vq + TCONV - 1) // TCONV
    pad = ntiles * TCONV - Vq
    if pad:
        spr = np.concatenate([spr, np.zeros((pad, L), np.int64)], 0)
    arr = spr.reshape(ntiles, G, P, L).transpose(0, 1, 3, 2)  # [t, gr, j, p]
    flat = arr.reshape(ntiles, 36 * P)
    blocks = [_wrap16(flat[t]) for t in range(ntiles)]  # [128, 289] each
    return np.concatenate(blocks, 1).astype(np.int16), ntiles


def preprocess(d):
    pr = {}
    for l in range(4):
        pr[f"pd{l}"] = _prep_pool(np.asarray(d[f"dr{l}"]), np.asarray(d[f"dc{l}"]),
                                  np.asarray(d[f"dv{l}"]), V[l + 1])
        pr[f"pu{l}"] = _prep_pool(np.asarray(d[f"ur{l}"]), np.asarray(d[f"uc{l}"]),
                                  np.asarray(d[f"uv{l}"]), V[l])
    sps = [np.asarray(d[f"sp{l}"]) for l in range(4)]
    # conv gather indices (int16-wrapped, per-tile blocks)
    for l in range(1, 4):
        pr[f"ce{l}"] = _conv_idx16(sps[l], pr[f"pd{l-1}"]["inv"])
    for l in range(4):
        pr[f"cd{l}"] = _conv_idx16(sps[l], pr[f"pu{l}"]["inv"])
    pr["cf"] = _conv_idx16(sps[0], None)
    # pool gather indices
    for pname in ["pd0", "pd1", "pd2", "pd3", "pu3", "pu2", "pu1", "pu0"]:
        pr[f"{pname}_g"] = _pool_idx16(pr[pname],
                                       brk=ROT // P if pname == "pu0" else None)
    # host x-gather layout for ce0: feature-major rhs [27, nt0*512]
    sp0 = sps[0].astype(np.int64)
    nt0 = (V[0] + TCONV - 1) // TCONV
    pad = nt0 * TCONV - V[0]
    sp0p = np.concatenate([sp0, np.zeros((pad, L), np.int64)], 0)
    pr["xg_gi"] = sp0p  # [nt0*512, 9]
    pr["nt0"] = nt0
    # final-conv y-gather idx: 20 groups x 2048 verts x 9 taps, wrap blocks
    sp0g = np.concatenate([sp0p, np.zeros((40960 - sp0p.shape[0], L), np.int64)], 0)
    blocks = []
    for gi2 in range(20):
        for j in range(L):
            blocks.append(_wrap16(sp0g[gi2 * 2048:(gi2 + 1) * 2048, j]))
    pr["cfy"] = np.concatenate(blocks, 1).astype(np.int16)
    Wenc = np.asarray(d["Wenc"]).reshape(V[4], 64, LAT)
    pr["Wenc_p"] = np.ascontiguousarray(
        Wenc[pr["pd3"]["perm"]].reshape(V[4] * 64, LAT))
    return pr


def build(pr, weights, dbg=False):
    from concourse import bacc, bass, tile, mybir
    from concourse.bass import exact_div
    import ml_dtypes

    f32, bf16, i32, i16 = (mybir.dt.float32, mybir.dt.bfloat16, mybir.dt.int32,
                           mybir.dt.int16)
    nc = bacc.Bacc(None, target_bir_lowering=False, num_swdge_queues=NQ)
    ext = {}
    qctr = [0]

    def qrr():
        q = qctr[0] % NQ
        qctr[0] += 1
        return q

    def dma_gather(out_ap, in_ap, idxs_ap, num_idxs, elem_size, elem_step):
        stride_bytes = elem_step * mybir.dt.size(in_ap.dtype)
        eng = nc.gpsimd
        return eng.add_instruction(
            mybir.InstDMAGatherAnt(
                name=eng.bass.get_next_instruction_name(),
                ins=[*eng.lower_ap_dma(in_ap, for_custom_bir_dma=True),
                     eng.lower_ap(idxs_ap),
                     eng.lower_val_access(eng.to_reg(num_idxs))],
                outs=[eng.lower_ap(out_ap)],
                transpose=False, num_idxs=num_idxs, elem_size=elem_size,
                stride_bytes_256=exact_div(stride_bytes, 256), gen_mode=0,
                single_packet=False, queue_num=qrr(),
                sbuf_tokens_per_rank=0, sbuf_free_dim_per_rank=0,
                sbuf_free_dim_pad_per_rank=0, sbuf_byte_offset=0))

    def ein(name, arr, dtype):
        h = nc.dram_tensor(name, list(arr.shape), dtype, kind="ExternalInput")
        if dtype == bf16:
            ext[name] = np.asarray(arr).astype(ml_dtypes.bfloat16)
        elif dtype == i32:
            ext[name] = np.asarray(arr).astype(np.int32)
        elif dtype == i16:
            ext[name] = np.asarray(arr).astype(np.int16)
        else:
            ext[name] = np.asarray(arr).astype(np.float32)
        return h

    nt0 = pr["nt0"]
    xg_h = nc.dram_tensor("xgT", [27, nt0 * TCONV], bf16, kind="ExternalInput")
    cfy_h = ein("cfy", pr["cfy"], i16)
    conv_idx = {}
    for cname in ["ce1", "ce2", "ce3", "cd3", "cd2", "cd1", "cd0", "cf"]:
        idx, nt = pr[cname]
        conv_idx[cname] = (ein(f"idx_{cname}", idx, i16), nt)
    pool_meta = {}
    for pname in ["pd0", "pd1", "pd2", "pd3", "pu3", "pu2", "pu1", "pu0"]:
        pp = pr[pname]
        chunks, idx16, offs = pr[f"{pname}_g"]
        pool_meta[pname] = dict(
            idx=ein(f"pidx_{pname}", idx16, i16),
            val=ein(f"pval_{pname}", pp["val"], f32),
            widths=pp["widths"], ntiles=pp["ntiles"], chunks=chunks, offs=offs)
    weights = dict(weights)
    weights["ident"] = np.eye(P, dtype=np.float32)
    weights["identb"] = np.eye(P, dtype=np.float32)
    wdict = {}
    for nm, arr in weights.items():
        wdict[nm] = ein(nm, arr, bf16 if (nm.startswith("W") or nm == "identb") else f32)

    # tables: [rows, 128] bf16 at 256B pitch (rot tables are 65536 rows)
    tabs = {}
    for nm, rows in dict(
            te0=VB, tp0=_tpad(V[1], P), te1=_tpad(V[1], TCONV),
            tp1=_tpad(V[2], P), te2=_tpad(V[2], TCONV), tp2=_tpad(V[3], P),
            te3=_tpad(V[3], TCONV), thd=V[4],
            tu3=_tpad(V[3], P), td3=_tpad(V[3], TCONV),
            tu2=_tpad(V[2], P), td2=_tpad(V[2], TCONV),
            tu1=_tpad(V[1], P), td1=_tpad(V[1], TCONV),
            tu0=VB, ty=VB).items():
        tabs[nm] = nc.dram_tensor(f"tab_{nm}", [rows, EP], bf16,
                                  kind="ExternalOutput" if dbg else "Internal")
    tp3_h = nc.dram_tensor("tab_tp3", [_tpad(V[4], P), 64], bf16,
                           kind="ExternalOutput" if dbg else "Internal")
    out_h = nc.dram_tensor("out", [V[0], 3], f32, kind="ExternalOutput")

    AT = mybir.ActivationFunctionType
    OP = mybir.AluOpType

    def src_view(tab, cin, rot):
        v3 = tab[:].rearrange("v (s c) -> v s c", c=cin)
        return v3[ROT:, 0, :] if rot else v3[:, 0, :]

    def rotpos(v0):
        return (v0 + ROT) % VB

    with tile.TileContext(nc) as tc:
        from contextlib import ExitStack
        es = ExitStack()
        wp = es.enter_context(tc.tile_pool(name="wp", bufs=1))
        gcv = es.enter_context(tc.tile_pool(name="gcv", bufs=8))
        gpl = es.enter_context(tc.tile_pool(name="gpl", bufs=5))
        gxp = es.enter_context(tc.tile_pool(name="gxp", bufs=2))
        ixp = es.enter_context(tc.tile_pool(name="ixp", bufs=4))
        rhp = es.enter_context(tc.tile_pool(name="rhp", bufs=7))
        elp = es.enter_context(tc.tile_pool(name="elp", bufs=2))
        pps = es.enter_context(tc.tile_pool(name="pps", bufs=2))
        stp = es.enter_context(tc.tile_pool(name="stp", bufs=3))
        pst = es.enter_context(tc.tile_pool(name="pst", bufs=2, space="PSUM"))
        pso = es.enter_context(tc.tile_pool(name="pso", bufs=1, space="PSUM"))

        idf = wp.tile([P, P], f32, tag="idf")
        nc.sync.dma_start(out=idf[:], in_=wdict["ident"][:])
        idb = wp.tile([P, P], bf16, tag="idb")
        nc.sync.dma_start(out=idb[:], in_=wdict["identb"][:])
        benc_sb = wp.tile([1, LAT], f32, tag="benc")
        nc.sync.dma_start(out=benc_sb[:], in_=wdict["benc"][None, :])

        wf9_sb = wp.tile([32, 36], bf16, tag="wf9")
        nc.sync.dma_start(out=wf9_sb[:], in_=wdict["Wf9"][:])
        wsb, bsb = {}, {}
        for nm in ["We0", "We1", "We2", "We3", "Wd0", "Wd1", "Wd2", "Wd3", "Wf"]:
            h = wdict[nm]
            rows, cout = h.shape
            nchunk = (rows + P - 1) // P
            t = wp.tile([min(P, rows), nchunk * cout], bf16, tag=f"w_{nm}")
            for ck in range(nchunk):
                r0, r1 = ck * P, min(rows, (ck + 1) * P)
                nc.sync.dma_start(out=t[: r1 - r0, ck * cout:(ck + 1) * cout],
                                  in_=h[r0:r1, :])
            wsb[nm] = (t, rows, cout)
        for nm in ["be0", "be1", "be2", "be3", "bd0", "bd1", "bd2", "bd3", "bf"]:
            h = wdict[nm]
            t = wp.tile([h.shape[0], 1], f32, tag=f"b_{nm}")
            nc.sync.dma_start(out=t[:], in_=h[:, None])
            bsb[nm] = t

        def conv(cname, src_tab, src_rot, cin, wname, bname, dst_tab, dst_rot,
                 Vq, elu=True, final=False, yfuse=False):
            YC = 36
            dense = cname == "ce0"
            if dense:
                ntiles = nt0
            else:
                idx_h, ntiles = conv_idx[cname]
            wt, wrows, cout = wsb[wname]
            bias = bsb[bname]
            spj = max(1, P // cin)
            nchunks = (L + spj - 1) // spj
            chunks = [(b * spj, min(L, (b + 1) * spj)) for b in range(nchunks)]
            NG = 4
            o_dt = f32 if final else bf16
            sview = None if dense else src_view(src_tab, cin, src_rot)
            ix_pre = {}

            def load_ix(c0p):
                if dense or c0p >= ntiles:
                    return
                ngp = min(NG, ntiles - c0p)
                ixn = ixp.tile([P, NG * 289], i16, tag="cidx", bufs=3,
                               name="ixn")
                nc.gpsimd.dma_start(out=ixn[:, : ngp * 289],
                                    in_=idx_h[:, c0p * 289:(c0p + ngp) * 289])
                ix_pre[c0p] = ixn

            load_ix(0)
            load_ix(NG)
            for c0 in range(0, ntiles, NG):
                ng = min(NG, ntiles - c0)
                npair = (ng + 1) // 2
                # --- inputs ---
                if dense:
                    gx = gxp.tile([27, NG * TCONV], bf16, tag="cgxT")
                    nc.gpsimd.dma_start(
                        out=gx[:, : ng * TCONV],
                        in_=xg_h[:, c0 * TCONV:(c0 + ng) * TCONV])
                    rhs = {0: gx}
                else:
                    ix = ix_pre.pop(c0)
                    gts = []
                    for t in range(ng):
                        g = gcv.tile([P, 37 * 64], bf16, tag="cg")
                        gv = g[:, : 37 * cin].rearrange("p (q c) -> p q c", c=cin)
                        dma_gather(gv, sview, ix[:, t * 289:(t + 1) * 289],
                                   36 * P + 16, cin, EP)
                        gts.append(g[:, : 36 * cin])
                    qrr()  # rotate each group's starting queue (breaks stagger lock)
                    load_ix(c0 + 2 * NG)
                    # phase T: transposes in tile-pairs (bf16 PSUM), DVE copy out
                    rhs = {}
                    for bI, (j0, j1) in enumerate(chunks):
                        cb = (j1 - j0) * cin
                        r = rhp.tile([P, NG * TCONV], bf16, tag="rhs")
                        for h in range(npair):
                            th = min(2, ng - 2 * h)
                            tp = pst.tile([P, 2 * TCONV], bf16, tag="tpb")
                            for ti in range(th):
                                gt = gts[2 * h + ti]
                                for gr in range(G):
                                    nc.tensor.transpose(
                                        out=tp[:cb, (ti * G + gr) * P:(ti * G + gr + 1) * P],
                                        in_=gt[:, (gr * L + j0) * cin:(gr * L + j1) * cin],
                                        identity=idb[:, :])
                            nc.vector.tensor_copy(
                                out=r[:cb, 2 * h * TCONV:(2 * h + th) * TCONV],
                                in_=tp[:cb, : th * TCONV])
                        rhs[bI] = r
                # phase M: one 4-bank PSUM tile for the whole group
                pos = pso.tile([64, NG * TCONV], f32, tag="po")
                for bI, (j0, j1) in enumerate(chunks):
                    cb = (j1 - j0) * cin
                    rsrc = rhs[0 if dense else bI]
                    for t in range(ng):
                        nc.tensor.matmul(
                            out=pos[:cout, t * TCONV:(t + 1) * TCONV],
                            lhsT=wt[:cb, bI * cout:(bI + 1) * cout],
                            rhs=rsrc[:cb, t * TCONV:(t + 1) * TCONV],
                            start=(bI == 0), stop=(bI == nchunks - 1))
                # phase E+B per pair: epilogue then back transposes
                if final:
                    tb = pst.tile([P, TCONV], f32, tag="tpm", bufs=1, name="tbf")
                else:
                    tb = pst.tile([P, 2 * TCONV], bf16, tag="tb", bufs=1, name="tbb")
                for h in range(npair):
                    th = min(2, ng - 2 * h)
                    nv5 = th * TCONV
                    pslc = pos[:cout, 2 * h * TCONV: 2 * h * TCONV + nv5]
                    if elu:
                        a1 = elp.tile([64, 2 * TCONV], f32, tag="a1")
                        nc.scalar.activation(out=a1[:cout, :nv5], in_=pslc,
                                             func=AT.Identity, bias=bias[:, :],
                                             scale=1.0)
                        m = elp.tile([64, 2 * TCONV], f32, tag="m")
                        nc.vector.tensor_scalar_min(out=m[:cout, :nv5],
                                                    in0=a1[:cout, :nv5], scalar1=0.0)
                        e = elp.tile([64, 2 * TCONV], f32, tag="e")
                        nc.scalar.activation(out=e[:cout, :nv5], in_=m[:cout, :nv5],
                                             func=AT.Exp)
                        src_o = elp.tile([64, 2 * TCONV], bf16, tag="ofin")
                        nc.vector.scalar_tensor_tensor(
                            out=src_o[:cout, :nv5], in0=e[:cout, :nv5], scalar=-1.0,
                            in1=a1[:cout, :nv5], op0=OP.add, op1=OP.max)
                    else:
                        src_o = elp.tile([64, 2 * TCONV], f32, tag="a1f")
                        nc.scalar.activation(out=src_o[:cout, :nv5], in_=pslc,
                                             func=AT.Identity, bias=bias[:, :],
                                             scale=1.0)
                    for ti in range(th):
                        t = 2 * h + ti
                        if yfuse:
                            yps = pst.tile([P, TCONV], f32, tag="tpm", bufs=1,
                                           name="yps")
                            nc.tensor.matmul(
                                out=yps[:YC, :],
                                lhsT=wf9_sb[:32, :YC],
                                rhs=src_o[:cout, ti * TCONV:(ti + 1) * TCONV],
                                start=True, stop=True)
                            ysb = elp.tile([40, TCONV], bf16, tag="ysb", bufs=2,
                                           name="ysb")
                            nc.scalar.copy(out=ysb[:YC, :], in_=yps[:YC, :])
                            for gr in range(G):
                                nc.tensor.transpose(
                                    out=tb[:, (t * G + gr) * YC:(t * G + gr + 1) * YC],
                                    in_=ysb[:YC, gr * P:(gr + 1) * P],
                                    identity=idb[:YC, :YC])
                            continue
                        for gr in range(G):
                            nc.tensor.transpose(
                                out=tb[:, (t * G + gr) * cout:(t * G + gr + 1) * cout],
                                in_=src_o[:cout, ti * TCONV + gr * P:
                                          ti * TCONV + (gr + 1) * P],
                                identity=(idf if final else idb)[:cout, :cout])
                cw = YC if yfuse else cout
                ost = stp.tile([P, NG * G * 64], o_dt,
                               tag=("costf" if final else "cost"),
                               bufs=(2 if final else 3))
                nc.vector.tensor_copy(out=ost[:, : ng * G * cw],
                                      in_=tb[:, : ng * G * cw])
                # write group (contiguous rows; split if the rotation wraps)
                v0 = c0 * TCONV
                if not final:
                    nrows = ng * TCONV
                    p0 = rotpos(v0) if dst_rot else v0
                    segs = ([(p0, 0, nrows)] if not dst_rot or p0 + nrows <= VB
                            else [(p0, 0, VB - p0), (0, VB - p0, nrows - (VB - p0))])
                    for (d0, s0, nr) in segs:
                        nc.sync.dma_start(
                            out=dst_tab[d0: d0 + nr, :cw]
                                .rearrange("(q p) c -> p q c", p=P),
                            in_=ost[:, s0 // P * cw: (s0 + nr) // P * cw]
                                .rearrange("p (q c) -> p q c", c=cw))
                else:
                    nv = min(Vq - v0, ng * TCONV)
                    full_q = nv // P
                    if full_q:
                        nc.sync.dma_start(
                            out=dst_tab[v0: v0 + full_q * P, :]
                                .rearrange("(q p) c -> p q c", p=P),
                            in_=ost[:, : full_q * cout]
                                .rearrange("p (q c) -> p q c", c=cout))
                    rem = nv - full_q * P
                    if rem:
                        nc.sync.dma_start(
                            out=dst_tab[v0 + full_q * P: v0 + nv, :],
                            in_=ost[:rem, full_q * cout:(full_q + 1) * cout])

        def pool(pname, src_tab, src_rot, C, dst_tab, dst_rot):
            meta = pool_meta[pname]
            idx_h, val_h = meta["idx"], meta["val"]
            chunks, offs = meta["chunks"], meta["offs"]
            sview = src_view(src_tab, C, src_rot)
            icol = 0
            for (t0, t1, wsum) in chunks:
                nt = t1 - t0
                W = wsum // nt
                ix = ixp.tile([P, POOL_WMAX * 8 + 1], i16, tag="pidx")
                vl = ixp.tile([P, POOL_WMAX], f32, tag="pval")
                nc.gpsimd.dma_start(out=ix[:, : wsum * 8 + 1],
                                    in_=idx_h[:, icol:icol + wsum * 8 + 1])
                nc.gpsimd.dma_start(out=vl[:, :wsum], in_=val_h[:, offs[t0]:offs[t1]])
                g = gpl.tile([P, (POOL_WMAX + 1) * 64], bf16, tag="pg")
                gv = g[:].rearrange("p (q c) -> p q c", c=C)[:, :wsum + 1, :]
                dma_gather(gv, sview, ix[:, : wsum * 8 + 1], wsum * P + 16, C, EP)
                s = pps.tile([P, POOL_WMAX * 64], f32, tag="ps")
                sv = s[:, : wsum * C].rearrange("p (t w c) -> p t w c", w=W, c=C)
                nc.vector.tensor_tensor(
                    out=sv,
                    in0=g[:, : wsum * C].rearrange("p (t w c) -> p t w c", w=W, c=C),
                    in1=vl[:, :wsum].rearrange("p (t w) -> p t w", w=W)[:, :, :, None]
                        .to_broadcast([P, nt, W, C]),
                    op=OP.mult)
                h = W
                while h > 1:
                    h //= 2
                    nc.vector.tensor_tensor(
                        out=sv[:, :, :h, :], in0=sv[:, :, :h, :],
                        in1=sv[:, :, h:2 * h, :], op=OP.add)
                ost = stp.tile([P, POOL_SUB * 64], bf16, tag="post")
                nc.vector.tensor_copy(out=ost[:, : nt * C], in_=sv[:, :, 0, :])
                p0 = rotpos(t0 * P) if dst_rot else t0 * P
                nc.sync.dma_start(
                    out=dst_tab[p0: p0 + nt * P, :C].rearrange("(q p) c -> p q c", p=P),
                    in_=ost[:, : nt * C].rearrange("p (q c) -> p q c", c=C))
                icol += wsum * 8 + 1

        # ---------------- network ----------------
        conv("ce0", None, False, 3, "We0", "be0", tabs["te0"], True, V[0])
        tc.strict_bb_all_engine_barrier()  # te0 rotated: gather range undeclared
        pool("pd0", tabs["te0"], True, 32, tabs["tp0"], False)
        conv("ce1", tabs["tp0"], False, 32, "We1", "be1", tabs["te1"], False, V[1])
        pool("pd1", tabs["te1"], False, 32, tabs["tp1"], False)
        conv("ce2", tabs["tp1"], False, 32, "We2", "be2", tabs["te2"], False, V[2])
        pool("pd2", tabs["te2"], False, 32, tabs["tp2"], False)
        conv("ce3", tabs["tp2"], False, 32, "We3", "be3", tabs["te3"], False, V[3])
        pool("pd3", tabs["te3"], False, 64, tp3_h, False)

        h4a = wp.tile([P, 64], bf16, tag="h4a")
        h4b = wp.tile([P, 64], bf16, tag="h4b")
        nc.sync.dma_start(out=h4a[:], in_=tp3_h[0:P, :])
        nc.sync.dma_start(out=h4b[:32, :], in_=tp3_h[P:160, :])
        h4T = wp.tile([64, 160], bf16, tag="h4T")
        tpa = pst.tile([P, TCONV], f32, tag="tpm", bufs=1)
        tpav = tpa[:].bitcast(bf16)
        nc.tensor.transpose(out=tpav[:64, 0:P], in_=h4a[:, :], identity=idb[:, :])
        nc.vector.tensor_copy(out=h4T[:, 0:P], in_=tpav[:64, 0:P])
        tpb = pst.tile([P, TCONV], f32, tag="tpm", bufs=1)
        tpbv = tpb[:].bitcast(bf16)
        nc.tensor.transpose(out=tpbv[:64, :32], in_=h4b[:32, :], identity=idb[:32, :32])
        nc.vector.tensor_copy(out=h4T[:, P:160], in_=tpbv[:64, :32])
        fl = wp.tile([P, 80], bf16, tag="fl")
        nc.vector.tensor_copy(out=fl[0:64, :], in_=h4T[:, 0:160:2])
        nc.vector.tensor_copy(out=fl[64:P, :], in_=h4T[:, 1:160:2])
        zps = pst.tile([P, TCONV], f32, tag="tpm", bufs=1)
        wev = wdict["Wenc_p"][:].rearrange("(k p) n -> p k n", p=P)
        for kc in range(8):
            wch = ixp.tile([P, 10 * LAT], bf16, tag="wenck", bufs=1)
            nc.gpsimd.dma_start(
                out=wch[:].rearrange("p (k n) -> p k n", n=LAT),
                in_=wev[:, kc * 10:(kc + 1) * 10, :])
            wv = wch[:].rearrange("p (k n) -> p k n", n=LAT)
            for k1 in range(10):
                k = kc * 10 + k1
                nc.tensor.matmul(out=zps[:1, :LAT], lhsT=fl[:, k:k + 1],
                                 rhs=wv[:, k1, :],
                                 start=(k == 0), stop=(k == 79))
        z_sb = wp.tile([1, LAT], bf16, tag="z_sb")
        nc.vector.tensor_tensor(out=z_sb[:], in0=zps[:1, :LAT], in1=benc_sb[:],
                                op=OP.add)
        zT = wp.tile([P, 2], bf16, tag="zT")
        for k2 in range(2):
            tz = pst.tile([P, TCONV], f32, tag="tpm", bufs=1)
            tzv = tz[:].bitcast(bf16)
            nc.tensor.transpose(out=tzv[:, 0:1], in_=z_sb[:, k2 * P:(k2 + 1) * P],
                                identity=idb[:1, :1])
            nc.vector.tensor_copy(out=zT[:, k2:k2 + 1], in_=tzv[:, 0:1])
        for n in range(20):
            wdc = ixp.tile([P, 2 * TCONV], bf16, tag="wdc", bufs=2)
            nc.gpsimd.dma_start(
                out=wdc[:].rearrange("p (k c) -> p k c", c=TCONV),
                in_=wdict["Wdec"][:, n * 512:(n + 1) * 512]
                    .rearrange("(k p) c -> p k c", p=P))
            hp = pst.tile([P, TCONV], f32, tag="tpm", bufs=1)
            for k2 in range(2):
                nc.tensor.matmul(out=hp[:1, :TCONV], lhsT=zT[:, k2:k2 + 1],
                                 rhs=wdc[:, k2 * TCONV:(k2 + 1) * TCONV],
                                 start=(k2 == 0), stop=(k2 == 1))
            bdc = ixp.tile([1, TCONV], f32, tag="bdc", bufs=1)
            nc.gpsimd.dma_start(out=bdc[:], in_=wdict["bdec"][None, n * 512:(n + 1) * 512])
            hdo = stp.tile([1, TCONV], bf16, tag="hdo", bufs=2)
            nc.vector.tensor_tensor(out=hdo[:], in0=hp[:1, :TCONV], in1=bdc[:],
                                    op=OP.add)
            nc.sync.dma_start(
                out=tabs["thd"][n * 8:(n + 1) * 8, :64],
                in_=hdo[:].rearrange("o (v c) -> o v c", c=64))

        pool("pu3", tabs["thd"], False, 64, tabs["tu3"], False)
        conv("cd3", tabs["tu3"], False, 64, "Wd3", "bd3", tabs["td3"], False, V[3])
        pool("pu2", tabs["td3"], False, 64, tabs["tu2"], False)
        conv("cd2", tabs["tu2"], False, 64, "Wd2", "bd2", tabs["td2"], False, V[2])
        pool("pu1", tabs["td2"], False, 32, tabs["tu1"], False)
        conv("cd1", tabs["tu1"], False, 32, "Wd1", "bd1", tabs["td1"], False, V[1])
        pool("pu0", tabs["td1"], False, 32, tabs["tu0"], True)
        tc.strict_bb_all_engine_barrier()  # tu0 rotated
        conv("cd0", tabs["tu0"], True, 32, "Wd0", "bd0", tabs["ty"], True, V[0],
             yfuse=True)
        tc.strict_bb_all_engine_barrier()  # ty rotated
        # final conv: ty holds y_j = ELU(h_d0) @ Wf_j per vertex (4-ch slots);
        # out[v] = sum_j y_j[sp0[v, j]] + bf via 8B-elem gathers + tree add
        bfr = wp.tile([P, 4], f32, tag="bfr")
        nc.sync.dma_start(out=bfr[:], in_=wdict["bf_rep"][:])
        tyt = tabs["ty"]
        for gi2 in range(20):
            v0 = gi2 * 2048
            ixf = ixp.tile([P, 9 * 129], i16, tag="cidx", bufs=3, name="ixf")
            nc.gpsimd.dma_start(out=ixf[:],
                                in_=cfy_h[:, gi2 * 1161:(gi2 + 1) * 1161])
            gy = ixp.tile([P, 9 * 68], bf16, tag="gy", bufs=4, name="gy")
            for j in range(L):
                gv = (gy[:, j * 68:(j + 1) * 68]
                      .rearrange("p (q c) -> p q c", c=4)[:, :17, :])
                dma_gather(gv, tyt[ROT:, j * 4:(j + 1) * 4],
                           ixf[:, j * 129:(j + 1) * 129], 2048 + 16, 4, EP)
            gyv = (gy[:].rearrange("p (j x) -> p j x", x=68)[:, :, :64]
                   .rearrange("p j (q c) -> p j q c", c=4))
            nc.vector.tensor_tensor(out=gyv[:, 0:4], in0=gyv[:, 0:4],
                                    in1=gyv[:, 4:8], op=OP.add)
            nc.vector.tensor_tensor(out=gyv[:, 0:2], in0=gyv[:, 0:2],
                                    in1=gyv[:, 2:4], op=OP.add)
            nc.vector.tensor_tensor(out=gyv[:, 0:1], in0=gyv[:, 0:1],
                                    in1=gyv[:, 1:2], op=OP.add)
            nc.vector.tensor_tensor(out=gyv[:, 0:1], in0=gyv[:, 0:1],
                                    in1=gyv[:, 8:9], op=OP.add)
            fout = stp.tile([P, 64], f32, tag="fout", bufs=2)
            nc.vector.tensor_tensor(
                out=fout[:].rearrange("p (q c) -> p q c", c=4),
                in0=gyv[:, 0, :, :],
                in1=bfr[:, None, :].to_broadcast([P, 16, 4]), op=OP.add)
            nv = min(V[0] - v0, 2048)
            full_q = nv // P
            if full_q:
                nc.sync.dma_start(
                    out=out_h[v0: v0 + full_q * P, :]
                        .rearrange("(q p) c -> p q c", p=P),
                    in_=fout[:].rearrange("p (q c) -> p q c", c=4)[:, :full_q, :3])
            rem = nv - full_q * P
            if rem:
                nc.sync.dma_start(
                    out=out_h[v0 + full_q * P: v0 + nv, :],
                    in_=fout[:rem, full_q * 4: full_q * 4 + 3])
        es.close()

    if not nc.is_finalized():
        nc.finalize()
    return nc, ext


def _weights_dict(d, pr):
    w = {}
    for l in range(4):
        for nm in (f"We{l}", f"be{l}", f"Wd{l}", f"bd{l}"):
            w[nm] = np.asarray(d[nm], np.float32)
    for nm in ("Wf", "bf", "benc", "Wdec", "bdec"):
        w[nm] = np.asarray(d[nm], np.float32)
    w["Wenc_p"] = np.asarray(pr["Wenc_p"], np.float32)
    bf = np.asarray(d["bf"], np.float32)
    w["bf_rep"] = np.concatenate(
        [np.tile(bf[None, :], (128, 1)), np.zeros((128, 1), np.float32)], 1)
    wf9 = np.zeros((32, 9, 4), np.float32)
    wf9[:, :, :3] = np.asarray(d["Wf"], np.float32).reshape(9, 32, 3).transpose(1, 0, 2)
    w["Wf9"] = np.ascontiguousarray(wf9.reshape(32, 36))
    return w


def _run(inputs, trace=False, **kw):
    import ml_dtypes
    d = {k: np.asarray(v) for k, v in inputs.items()}
    pr = preprocess(d)
    nc, ext = build(pr, _weights_dict(d, pr))
    from concourse.bass_utils import run_bass_kernel_spmd
    gi = pr["xg_gi"]  # [nt0*512, 9]
    in_maps = []
    for i in range(B):
        m = dict(ext)
        xb = np.asarray(d["x"][i], np.float32)
        xg = xb[gi].reshape(-1, 27)  # [nt0*512, 27]
        m["xgT"] = np.ascontiguousarray(xg.T).astype(ml_dtypes.bfloat16)
        in_maps.append(m)
    res = run_bass_kernel_spmd(nc, in_maps, core_ids=list(range(B)),
                               trace=trace, **kw)
    out = np.stack([np.asarray(r["out"], np.float32) for r in res.results], 0)
    return out, res


def kernel(**inputs):
    return _run(inputs)[0]


# revision 26
# speedup vs baseline: 1.2001x; 1.2001x over previous
"""Trainium2 Bass kernel for the spiral-conv mesh autoencoder (nn_AE_45810121179173).

Data-parallel over batch: core i runs the full network for batch element i.
- gathers via InstDMAGatherAnt (SWDGE dma_gather) round-robined over 4 SWDGE
  queues; per-queue throughput ~8ns/idx (64B elems) so saturation needs 4
  concurrent gather instructions -> small uniform chunks + deep buffering.
- tables stored at 256B pitch ([*, 128] bf16) as dma_gather requires the
  source pitch to be a multiple of 256B; elem_size = real channel count.
- indices are int16; 40000-row tables use a rotated 65536-row buffer (row v
  stored at (v+32768)%65536, gather base view at +32768) so uint16-wrapped
  indices address the full range via signed-offset wraparound.
- first conv's x-gather is done host-side and laid out feature-major, so
  conv ce0 needs no gather and no PE transposes.
- pools: host-side degree-sorted padded-CSR gather; chunks group equal-width
  tiles so the weighted tree-sum is one DVE op per halving level per chunk.
- convs: gather (vertex-major) -> PE transpose to feature-major (PSUM f32,
  two tiles per bank-pair) -> scalar-engine copy to SBUF bf16 -> PSUM-
  accumulated matmuls -> pair-wide bias + ELU epilogue (scalar+DVE bf16)
  -> PE transpose back -> single table write per group.
Self-contained: hardcodes shapes; needs numpy + concourse runtime only.
"""
import sys

for _p in ("/opt/trn_rl_repo", "/root/.axon_site/_ro/trn_rl_repo"):
    if _p not in sys.path:
        sys.path.insert(0, _p)

import numpy as np

V = [40000, 10000, 2500, 625, 160]
L = 9
LAT = 256
B = 8
P = 128
TCONV = 512
G = 4
EP = 128          # table pitch in bf16 elements (256B)
ROT = 32768       # rotation for 40000-row tables
VB = 65536        # rotated table rows
NQ = 4            # SWDGE queues
POOL_SUB = 32     # target gather columns per pool chunk (32*128=4096 idx)
POOL_WMAX = 32    # a single tile may be up to this wide


def _pow2_up(x):
    n = 1
    while n < x:
        n *= 2
    return n


def _tpad(v, m):
    return ((v + m - 1) // m) * m


def _wrap16(flat):
    """[n] values -> [128, n//16+1] int16 FW layout; appends a positive
    sentinel column so the FW's trailing-negative-pad scan never drops real
    entries whose uint16 wrap is negative."""
    n = len(flat)
    assert n % 16 == 0
    u = np.asarray(flat).astype(np.uint16).astype(np.int16)
    u = np.concatenate([u, np.zeros(16, np.int16)])
    w = u.reshape(n // 16 + 1, 16).T  # [16, cols]; entry i at [i%16, i//16]
    return np.tile(w, (8, 1))


def _prep_pool(row, col, val, v_out):
    nnz = len(row)
    deg = np.bincount(row, minlength=v_out)
    perm = np.argsort(-deg, kind="stable").astype(np.int64)
    inv = np.empty(v_out, np.int64)
    inv[perm] = np.arange(v_out)
    order = np.argsort(row, kind="stable")
    col_s, val_s = col[order].astype(np.int64), np.asarray(val)[order].astype(np.float32)
    starts = np.zeros(v_out + 1, np.int64)
    np.cumsum(deg, out=starts[1:])
    ntiles = (v_out + P - 1) // P
    widths, idx_l, val_l = [], [], []
    for t in range(ntiles):
        dests = perm[t * P:(t + 1) * P]
        k = deg[dests]
        W = _pow2_up(max(1, int(k.max()) if len(k) else 1))
        assert W <= POOL_WMAX, f"pool tile width {W} exceeds {POOL_WMAX}"
        npd = len(dests)
        mask = np.arange(W)[None, :] < k[:, None]
        flat = np.minimum(starts[dests][:, None] + np.arange(W)[None, :], max(nnz - 1, 0))
        I = np.zeros((P, W), np.int64)
        A = np.zeros((P, W), np.float32)
        I[:npd] = np.where(mask, col_s[flat], 0)
        A[:npd] = np.where(mask, val_s[flat], 0.0)
        widths.append(W)
        idx_l.append(I)
        val_l.append(A)
    return dict(perm=perm, inv=inv, widths=widths,
                idx=np.concatenate(idx_l, 1),
                val=np.concatenate(val_l, 1).astype(np.float32), ntiles=ntiles)


def _pool_chunks(widths, brk=None):
    """Uniform-W chunking: consecutive equal-width tiles, wsum <= POOL_SUB,
    break at tile brk. A single tile wider than POOL_SUB gets its own chunk."""
    chunks = []
    t0 = 0
    ntiles = len(widths)
    while t0 < ntiles:
        W = widths[t0]
        t1, wsum = t0, 0
        while (t1 < ntiles and widths[t1] == W and wsum + W <= POOL_SUB
               and not (brk is not None and t1 == brk and t1 > t0)):
            wsum += W
            t1 += 1
        if t1 == t0:  # W > POOL_SUB
            t1, wsum = t0 + 1, W
        chunks.append((t0, t1, wsum))
        t0 = t1
    return chunks


def _pool_idx16(pp, brk=None):
    """Build per-chunk wrapped idx + chunk list; idx flat order i=col*128+p."""
    widths = pp["widths"]
    chunks = _pool_chunks(widths, brk)
    offs = np.concatenate([[0], np.cumsum(widths)]).astype(int)
    blocks = []
    for (t0, t1, wsum) in chunks:
        cols = pp["idx"][:, offs[t0]:offs[t1]]  # [128, wsum]
        blocks.append(_wrap16(cols.T.reshape(-1)))  # [128, wsum*8+1]
    return chunks, np.concatenate(blocks, 1).astype(np.int16), offs


def _conv_idx16(sp, inv):
    """[128, ntiles*290] int16: per-tile TWO wrap half-blocks (gr01 | gr23),
    each [128, 145]; within a half i=(grh*9+j)*128+p."""
    Vq = sp.shape[0]
    spr = (inv[sp] if inv is not None else np.asarray(sp).astype(np.int64))
    ntiles = (Vq + TCONV - 1) // TCONV
    pad = ntiles * TCONV - Vq
    if pad:
        spr = np.concatenate([spr, np.zeros((pad, L), np.int64)], 0)
    arr = spr.reshape(ntiles, G, P, L).transpose(0, 1, 3, 2)  # [t, gr, j, p]
    blocks = []
    for t in range(ntiles):
        blocks.append(_wrap16(arr[t, :2].reshape(18 * P)))  # [128, 145]
        blocks.append(_wrap16(arr[t, 2:].reshape(18 * P)))
    return np.concatenate(blocks, 1).astype(np.int16), ntiles


def preprocess(d):
    pr = {}
    for l in range(4):
        pr[f"pd{l}"] = _prep_pool(np.asarray(d[f"dr{l}"]), np.asarray(d[f"dc{l}"]),
                                  np.asarray(d[f"dv{l}"]), V[l + 1])
        pr[f"pu{l}"] = _prep_pool(np.asarray(d[f"ur{l}"]), np.asarray(d[f"uc{l}"]),
                                  np.asarray(d[f"uv{l}"]), V[l])
    sps = [np.asarray(d[f"sp{l}"]) for l in range(4)]
    # conv gather indices (int16-wrapped, per-tile blocks)
    for l in range(1, 4):
        pr[f"ce{l}"] = _conv_idx16(sps[l], pr[f"pd{l-1}"]["inv"])
    for l in range(4):
        pr[f"cd{l}"] = _conv_idx16(sps[l], pr[f"pu{l}"]["inv"])
    pr["cf"] = _conv_idx16(sps[0], None)
    # pool gather indices
    for pname in ["pd0", "pd1", "pd2", "pd3", "pu3", "pu2", "pu1", "pu0"]:
        pr[f"{pname}_g"] = _pool_idx16(pr[pname],
                                       brk=ROT // P if pname == "pu0" else None)
    # host x-gather layout for ce0: feature-major rhs [27, nt0*512]
    sp0 = sps[0].astype(np.int64)
    nt0 = (V[0] + TCONV - 1) // TCONV
    pad = nt0 * TCONV - V[0]
    sp0p = np.concatenate([sp0, np.zeros((pad, L), np.int64)], 0)
    pr["xg_gi"] = sp0p  # [nt0*512, 9]
    pr["nt0"] = nt0
    # final-conv y-gather idx: 20 groups x 2048 verts x 9 taps, wrap blocks
    sp0g = np.concatenate([sp0p, np.zeros((40960 - sp0p.shape[0], L), np.int64)], 0)
    blocks = []
    for gi2 in range(20):
        for j in range(L):
            blocks.append(_wrap16(sp0g[gi2 * 2048:(gi2 + 1) * 2048, j]))
    pr["cfy"] = np.concatenate(blocks, 1).astype(np.int16)
    Wenc = np.asarray(d["Wenc"]).reshape(V[4], 64, LAT)
    pr["Wenc_p"] = np.ascontiguousarray(
        Wenc[pr["pd3"]["perm"]].reshape(V[4] * 64, LAT))
    return pr


def build(pr, weights, dbg=False):
    from concourse import bacc, bass, tile, mybir
    from concourse.bass import exact_div
    import ml_dtypes

    f32, bf16, i32, i16 = (mybir.dt.float32, mybir.dt.bfloat16, mybir.dt.int32,
                           mybir.dt.int16)
    nc = bacc.Bacc(None, target_bir_lowering=False, num_swdge_queues=NQ)
    ext = {}
    qctr = [0]

    def qrr():
        q = qctr[0] % NQ
        qctr[0] += 1
        return q

    def dma_gather(out_ap, in_ap, idxs_ap, num_idxs, elem_size, elem_step):
        stride_bytes = elem_step * mybir.dt.size(in_ap.dtype)
        eng = nc.gpsimd
        return eng.add_instruction(
            mybir.InstDMAGatherAnt(
                name=eng.bass.get_next_instruction_name(),
                ins=[*eng.lower_ap_dma(in_ap, for_custom_bir_dma=True),
                     eng.lower_ap(idxs_ap),
                     eng.lower_val_access(eng.to_reg(num_idxs))],
                outs=[eng.lower_ap(out_ap)],
                transpose=False, num_idxs=num_idxs, elem_size=elem_size,
                stride_bytes_256=exact_div(stride_bytes, 256), gen_mode=0,
                single_packet=False, queue_num=qrr(),
                sbuf_tokens_per_rank=0, sbuf_free_dim_per_rank=0,
                sbuf_free_dim_pad_per_rank=0, sbuf_byte_offset=0))

    def ein(name, arr, dtype):
        h = nc.dram_tensor(name, list(arr.shape), dtype, kind="ExternalInput")
        if dtype == bf16:
            ext[name] = np.asarray(arr).astype(ml_dtypes.bfloat16)
        elif dtype == i32:
            ext[name] = np.asarray(arr).astype(np.int32)
        elif dtype == i16:
            ext[name] = np.asarray(arr).astype(np.int16)
        else:
            ext[name] = np.asarray(arr).astype(np.float32)
        return h

    nt0 = pr["nt0"]
    xg_h = nc.dram_tensor("xgT", [27, nt0 * TCONV], bf16, kind="ExternalInput")
    cfy_h = ein("cfy", pr["cfy"], i16)
    conv_idx = {}
    for cname in ["ce1", "ce2", "ce3", "cd3", "cd2", "cd1", "cd0", "cf"]:
        idx, nt = pr[cname]
        conv_idx[cname] = (ein(f"idx_{cname}", idx, i16), nt)
    pool_meta = {}
    for pname in ["pd0", "pd1", "pd2", "pd3", "pu3", "pu2", "pu1", "pu0"]:
        pp = pr[pname]
        chunks, idx16, offs = pr[f"{pname}_g"]
        pool_meta[pname] = dict(
            idx=ein(f"pidx_{pname}", idx16, i16),
            val=ein(f"pval_{pname}", pp["val"], f32),
            widths=pp["widths"], ntiles=pp["ntiles"], chunks=chunks, offs=offs)
    weights = dict(weights)
    weights["ident"] = np.eye(P, dtype=np.float32)
    weights["identb"] = np.eye(P, dtype=np.float32)
    wdict = {}
    for nm, arr in weights.items():
        wdict[nm] = ein(nm, arr, bf16 if (nm.startswith("W") or nm == "identb") else f32)

    # tables: [rows, 128] bf16 at 256B pitch (rot tables are 65536 rows)
    tabs = {}
    for nm, rows in dict(
            te0=VB, tp0=_tpad(V[1], P), te1=_tpad(V[1], TCONV),
            tp1=_tpad(V[2], P), te2=_tpad(V[2], TCONV), tp2=_tpad(V[3], P),
            te3=_tpad(V[3], TCONV), thd=V[4],
            tu3=_tpad(V[3], P), td3=_tpad(V[3], TCONV),
            tu2=_tpad(V[2], P), td2=_tpad(V[2], TCONV),
            tu1=_tpad(V[1], P), td1=_tpad(V[1], TCONV),
            tu0=VB, ty=VB).items():
        tabs[nm] = nc.dram_tensor(f"tab_{nm}", [rows, EP], bf16,
                                  kind="ExternalOutput" if dbg else "Internal")
    tp3_h = nc.dram_tensor("tab_tp3", [_tpad(V[4], P), 64], bf16,
                           kind="ExternalOutput" if dbg else "Internal")
    out_h = nc.dram_tensor("out", [V[0], 3], f32, kind="ExternalOutput")

    AT = mybir.ActivationFunctionType
    OP = mybir.AluOpType

    def src_view(tab, cin, rot):
        v3 = tab[:].rearrange("v (s c) -> v s c", c=cin)
        return v3[ROT:, 0, :] if rot else v3[:, 0, :]

    def rotpos(v0):
        return (v0 + ROT) % VB

    with tile.TileContext(nc) as tc:
        from contextlib import ExitStack
        es = ExitStack()
        wp = es.enter_context(tc.tile_pool(name="wp", bufs=1))
        gcv = es.enter_context(tc.tile_pool(name="gcv", bufs=8))
        gpl = es.enter_context(tc.tile_pool(name="gpl", bufs=5))
        gxp = es.enter_context(tc.tile_pool(name="gxp", bufs=2))
        ixp = es.enter_context(tc.tile_pool(name="ixp", bufs=4))
        rhp = es.enter_context(tc.tile_pool(name="rhp", bufs=5))
        elp = es.enter_context(tc.tile_pool(name="elp", bufs=2))
        pps = es.enter_context(tc.tile_pool(name="pps", bufs=2))
        stp = es.enter_context(tc.tile_pool(name="stp", bufs=3))
        pst = es.enter_context(tc.tile_pool(name="pst", bufs=2, space="PSUM"))
        pso = es.enter_context(tc.tile_pool(name="pso", bufs=1, space="PSUM"))

        idf = wp.tile([P, P], f32, tag="idf")
        nc.sync.dma_start(out=idf[:], in_=wdict["ident"][:])
        idb = wp.tile([P, P], bf16, tag="idb")
        nc.sync.dma_start(out=idb[:], in_=wdict["identb"][:])
        benc_sb = wp.tile([1, LAT], f32, tag="benc")
        nc.sync.dma_start(out=benc_sb[:], in_=wdict["benc"][None, :])

        wf9_sb = wp.tile([32, 36], bf16, tag="wf9")
        nc.sync.dma_start(out=wf9_sb[:], in_=wdict["Wf9"][:])
        wsb, bsb = {}, {}
        for nm in ["We0", "We1", "We2", "We3", "Wd0", "Wd1", "Wd2", "Wd3", "Wf"]:
            h = wdict[nm]
            rows, cout = h.shape
            nchunk = (rows + P - 1) // P
            t = wp.tile([min(P, rows), nchunk * cout], bf16, tag=f"w_{nm}")
            for ck in range(nchunk):
                r0, r1 = ck * P, min(rows, (ck + 1) * P)
                nc.sync.dma_start(out=t[: r1 - r0, ck * cout:(ck + 1) * cout],
                                  in_=h[r0:r1, :])
            wsb[nm] = (t, rows, cout)
        for nm in ["be0", "be1", "be2", "be3", "bd0", "bd1", "bd2", "bd3", "bf"]:
            h = wdict[nm]
            t = wp.tile([h.shape[0], 1], f32, tag=f"b_{nm}")
            nc.sync.dma_start(out=t[:], in_=h[:, None])
            bsb[nm] = t

        def conv(cname, src_tab, src_rot, cin, wname, bname, dst_tab, dst_rot,
                 Vq, elu=True, final=False, yfuse=False):
            YC = 36
            dense = cname == "ce0"
            if dense:
                ntiles = nt0
            else:
                idx_h, ntiles = conv_idx[cname]
            wt, wrows, cout = wsb[wname]
            bias = bsb[bname]
            spj = max(1, P // cin)
            nchunks = (L + spj - 1) // spj
            chunks = [(b * spj, min(L, (b + 1) * spj)) for b in range(nchunks)]
            NG = 4
            o_dt = f32 if final else bf16
            sview = None if dense else src_view(src_tab, cin, src_rot)
            ix_pre = {}

            def load_ix(c0p):
                if dense or c0p >= ntiles:
                    return
                ngp = min(NG, ntiles - c0p)
                ixn = ixp.tile([P, NG * 290], i16, tag="cidx", bufs=3,
                               name="ixn")
                nc.gpsimd.dma_start(out=ixn[:, : ngp * 290],
                                    in_=idx_h[:, c0p * 290:(c0p + ngp) * 290])
                ix_pre[c0p] = ixn

            load_ix(0)
            load_ix(NG)
            for c0 in range(0, ntiles, NG):
                ng = min(NG, ntiles - c0)
                npair = (ng + 1) // 2
                # --- inputs ---
                if dense:
                    gx = gxp.tile([27, NG * TCONV], bf16, tag="cgxT")
                    nc.gpsimd.dma_start(
                        out=gx[:, : ng * TCONV],
                        in_=xg_h[:, c0 * TCONV:(c0 + ng) * TCONV])
                    rhs = {0: gx}
                else:
                    ix = ix_pre.pop(c0)
                    gts = []
                    for t in range(ng):
                        g = gcv.tile([P, 38 * 64], bf16, tag="cg")
                        for hh in range(2):
                            gv = (g[:, hh * 19 * cin: (hh * 19 + 19) * cin]
                                  .rearrange("p (q c) -> p q c", c=cin))
                            dma_gather(gv, sview,
                                       ix[:, t * 290 + hh * 145:
                                          t * 290 + (hh + 1) * 145],
                                       18 * P + 16, cin, EP)
                        gts.append(g)
                    load_ix(c0 + 2 * NG)
                    # phase T: transposes in tile-pairs (bf16 PSUM), DVE copy out
                    rhs = {}
                    for bI, (j0, j1) in enumerate(chunks):
                        cb = (j1 - j0) * cin
                        r = rhp.tile([P, NG * TCONV], bf16, tag="rhs")
                        for h in range(npair):
                            th = min(2, ng - 2 * h)
                            tp = pst.tile([P, 2 * TCONV], bf16, tag="tpb")
                            for ti in range(th):
                                gt = gts[2 * h + ti]
                                for gr in range(G):
                                    sl = (gr * L + j0 if gr < 2
                                          else 19 + (gr - 2) * L + j0)
                                    nc.tensor.transpose(
                                        out=tp[:cb, (ti * G + gr) * P:(ti * G + gr + 1) * P],
                                        in_=gt[:, sl * cin:(sl + j1 - j0) * cin],
                                        identity=idb[:, :])
                            nc.vector.tensor_copy(
                                out=r[:cb, 2 * h * TCONV:(2 * h + th) * TCONV],
                                in_=tp[:cb, : th * TCONV])
                        rhs[bI] = r
                # phase M: one 4-bank PSUM tile for the whole group
                pos = pso.tile([64, NG * TCONV], f32, tag="po")
                for bI, (j0, j1) in enumerate(chunks):
                    cb = (j1 - j0) * cin
                    rsrc = rhs[0 if dense else bI]
                    for t in range(ng):
                        nc.tensor.matmul(
                            out=pos[:cout, t * TCONV:(t + 1) * TCONV],
                            lhsT=wt[:cb, bI * cout:(bI + 1) * cout],
                            rhs=rsrc[:cb, t * TCONV:(t + 1) * TCONV],
                            start=(bI == 0), stop=(bI == nchunks - 1))
                # phase E+B per pair: epilogue then back transposes
                if final:
                    tb = pst.tile([P, TCONV], f32, tag="tpm", bufs=1, name="tbf")
                else:
                    tb = pst.tile([P, 2 * TCONV], bf16, tag="tb", bufs=1, name="tbb")
                for h in range(npair):
                    th = min(2, ng - 2 * h)
                    nv5 = th * TCONV
                    pslc = pos[:cout, 2 * h * TCONV: 2 * h * TCONV + nv5]
                    if elu:
                        a1 = elp.tile([64, 2 * TCONV], f32, tag="a1")
                        nc.scalar.activation(out=a1[:cout, :nv5], in_=pslc,
                                             func=AT.Identity, bias=bias[:, :],
                                             scale=1.0)
                        m = elp.tile([64, 2 * TCONV], f32, tag="m")
                        nc.vector.tensor_scalar_min(out=m[:cout, :nv5],
                                                    in0=a1[:cout, :nv5], scalar1=0.0)
                        e = elp.tile([64, 2 * TCONV], f32, tag="e")
                        nc.scalar.activation(out=e[:cout, :nv5], in_=m[:cout, :nv5],
                                             func=AT.Exp)
                        src_o = elp.tile([64, 2 * TCONV], bf16, tag="ofin")
                        nc.vector.scalar_tensor_tensor(
                            out=src_o[:cout, :nv5], in0=e[:cout, :nv5], scalar=-1.0,
                            in1=a1[:cout, :nv5], op0=OP.add, op1=OP.max)
                    else:
                        src_o = elp.tile([64, 2 * TCONV], f32, tag="a1f")
                        nc.scalar.activation(out=src_o[:cout, :nv5], in_=pslc,
                                             func=AT.Identity, bias=bias[:, :],
                                             scale=1.0)
                    for ti in range(th):
                        t = 2 * h + ti
                        if yfuse:
                            yps = pst.tile([P, TCONV], f32, tag="tpm", bufs=1,
                                           name="yps")
                            nc.tensor.matmul(
                                out=yps[:YC, :],
                                lhsT=wf9_sb[:32, :YC],
                                rhs=src_o[:cout, ti * TCONV:(ti + 1) * TCONV],
                                start=True, stop=True)
                            ysb = elp.tile([40, TCONV], bf16, tag="ysb", bufs=2,
                                           name="ysb")
                            nc.scalar.copy(out=ysb[:YC, :], in_=yps[:YC, :])
                            for gr in range(G):
                                nc.tensor.transpose(
                                    out=tb[:, (t * G + gr) * YC:(t * G + gr + 1) * YC],
                                    in_=ysb[:YC, gr * P:(gr + 1) * P],
                                    identity=idb[:YC, :YC])
                            continue
                        for gr in range(G):
                            nc.tensor.transpose(
                                out=tb[:, (t * G + gr) * cout:(t * G + gr + 1) * cout],
                                in_=src_o[:cout, ti * TCONV + gr * P:
                                          ti * TCONV + (gr + 1) * P],
                                identity=(idf if final else idb)[:cout, :cout])
                cw = YC if yfuse else cout
                ost = stp.tile([P, NG * G * 64], o_dt,
                               tag=("costf" if final else "cost"),
                               bufs=(2 if final else 3))
                nc.vector.tensor_copy(out=ost[:, : ng * G * cw],
                                      in_=tb[:, : ng * G * cw])
                # write group (contiguous rows; split if the rotation wraps)
                v0 = c0 * TCONV
                if not final:
                    nrows = ng * TCONV
                    p0 = rotpos(v0) if dst_rot else v0
                    segs = ([(p0, 0, nrows)] if not dst_rot or p0 + nrows <= VB
                            else [(p0, 0, VB - p0), (0, VB - p0, nrows - (VB - p0))])
                    for (d0, s0, nr) in segs:
                        nc.sync.dma_start(
                            out=dst_tab[d0: d0 + nr, :cw]
                                .rearrange("(q p) c -> p q c", p=P),
                            in_=ost[:, s0 // P * cw: (s0 + nr) // P * cw]
                                .rearrange("p (q c) -> p q c", c=cw))
                else:
                    nv = min(Vq - v0, ng * TCONV)
                    full_q = nv // P
                    if full_q:
                        nc.sync.dma_start(
                            out=dst_tab[v0: v0 + full_q * P, :]
                                .rearrange("(q p) c -> p q c", p=P),
                            in_=ost[:, : full_q * cout]
                                .rearrange("p (q c) -> p q c", c=cout))
                    rem = nv - full_q * P
                    if rem:
                        nc.sync.dma_start(
                            out=dst_tab[v0 + full_q * P: v0 + nv, :],
                            in_=ost[:rem, full_q * cout:(full_q + 1) * cout])

        def pool(pname, src_tab, src_rot, C, dst_tab, dst_rot):
            meta = pool_meta[pname]
            idx_h, val_h = meta["idx"], meta["val"]
            chunks, offs = meta["chunks"], meta["offs"]
            sview = src_view(src_tab, C, src_rot)
            icol = 0
            for (t0, t1, wsum) in chunks:
                nt = t1 - t0
                W = wsum // nt
                ix = ixp.tile([P, POOL_WMAX * 8 + 1], i16, tag="pidx")
                vl = ixp.tile([P, POOL_WMAX], f32, tag="pval")
                nc.gpsimd.dma_start(out=ix[:, : wsum * 8 + 1],
                                    in_=idx_h[:, icol:icol + wsum * 8 + 1])
                nc.gpsimd.dma_start(out=vl[:, :wsum], in_=val_h[:, offs[t0]:offs[t1]])
                g = gpl.tile([P, (POOL_WMAX + 1) * 64], bf16, tag="pg")
                gv = g[:].rearrange("p (q c) -> p q c", c=C)[:, :wsum + 1, :]
                dma_gather(gv, sview, ix[:, : wsum * 8 + 1], wsum * P + 16, C, EP)
                s = pps.tile([P, POOL_WMAX * 64], f32, tag="ps")
                sv = s[:, : wsum * C].rearrange("p (t w c) -> p t w c", w=W, c=C)
                nc.vector.tensor_tensor(
                    out=sv,
                    in0=g[:, : wsum * C].rearrange("p (t w c) -> p t w c", w=W, c=C),
                    in1=vl[:, :wsum].rearrange("p (t w) -> p t w", w=W)[:, :, :, None]
                        .to_broadcast([P, nt, W, C]),
                    op=OP.mult)
                h = W
                while h > 1:
                    h //= 2
                    nc.vector.tensor_tensor(
                        out=sv[:, :, :h, :], in0=sv[:, :, :h, :],
                        in1=sv[:, :, h:2 * h, :], op=OP.add)
                ost = stp.tile([P, POOL_SUB * 64], bf16, tag="post")
                nc.vector.tensor_copy(out=ost[:, : nt * C], in_=sv[:, :, 0, :])
                p0 = rotpos(t0 * P) if dst_rot else t0 * P
                nc.sync.dma_start(
                    out=dst_tab[p0: p0 + nt * P, :C].rearrange("(q p) c -> p q c", p=P),
                    in_=ost[:, : nt * C].rearrange("p (q c) -> p q c", c=C))
                icol += wsum * 8 + 1

        # ---------------- network ----------------
        conv("ce0", None, False, 3, "We0", "be0", tabs["te0"], True, V[0])
        tc.strict_bb_all_engine_barrier()  # te0 rotated: gather range undeclared
        pool("pd0", tabs["te0"], True, 32, tabs["tp0"], False)
        conv("ce1", tabs["tp0"], False, 32, "We1", "be1", tabs["te1"], False, V[1])
        pool("pd1", tabs["te1"], False, 32, tabs["tp1"], False)
        conv("ce2", tabs["tp1"], False, 32, "We2", "be2", tabs["te2"], False, V[2])
        pool("pd2", tabs["te2"], False, 32, tabs["tp2"], False)
        conv("ce3", tabs["tp2"], False, 32, "We3", "be3", tabs["te3"], False, V[3])
        pool("pd3", tabs["te3"], False, 64, tp3_h, False)

        h4a = wp.tile([P, 64], bf16, tag="h4a")
        h4b = wp.tile([P, 64], bf16, tag="h4b")
        nc.sync.dma_start(out=h4a[:], in_=tp3_h[0:P, :])
        nc.sync.dma_start(out=h4b[:32, :], in_=tp3_h[P:160, :])
        h4T = wp.tile([64, 160], bf16, tag="h4T")
        tpa = pst.tile([P, TCONV], f32, tag="tpm", bufs=1)
        tpav = tpa[:].bitcast(bf16)
        nc.tensor.transpose(out=tpav[:64, 0:P], in_=h4a[:, :], identity=idb[:, :])
        nc.vector.tensor_copy(out=h4T[:, 0:P], in_=tpav[:64, 0:P])
        tpb = pst.tile([P, TCONV], f32, tag="tpm", bufs=1)
        tpbv = tpb[:].bitcast(bf16)
        nc.tensor.transpose(out=tpbv[:64, :32], in_=h4b[:32, :], identity=idb[:32, :32])
        nc.vector.tensor_copy(out=h4T[:, P:160], in_=tpbv[:64, :32])
        fl = wp.tile([P, 80], bf16, tag="fl")
        nc.vector.tensor_copy(out=fl[0:64, :], in_=h4T[:, 0:160:2])
        nc.vector.tensor_copy(out=fl[64:P, :], in_=h4T[:, 1:160:2])
        zps = pst.tile([P, TCONV], f32, tag="tpm", bufs=1)
        wev = wdict["Wenc_p"][:].rearrange("(k p) n -> p k n", p=P)
        for kc in range(8):
            wch = ixp.tile([P, 10 * LAT], bf16, tag="wenck", bufs=2)
            nc.gpsimd.dma_start(
                out=wch[:].rearrange("p (k n) -> p k n", n=LAT),
                in_=wev[:, kc * 10:(kc + 1) * 10, :])
            wv = wch[:].rearrange("p (k n) -> p k n", n=LAT)
            for k1 in range(10):
                k = kc * 10 + k1
                nc.tensor.matmul(out=zps[:1, :LAT], lhsT=fl[:, k:k + 1],
                                 rhs=wv[:, k1, :],
                                 start=(k == 0), stop=(k == 79))
        z_sb = wp.tile([1, LAT], bf16, tag="z_sb")
        nc.vector.tensor_tensor(out=z_sb[:], in0=zps[:1, :LAT], in1=benc_sb[:],
                                op=OP.add)
        zT = wp.tile([P, 2], bf16, tag="zT")
        for k2 in range(2):
            tz = pst.tile([P, TCONV], f32, tag="tpm", bufs=1)
            tzv = tz[:].bitcast(bf16)
            nc.tensor.transpose(out=tzv[:, 0:1], in_=z_sb[:, k2 * P:(k2 + 1) * P],
                                identity=idb[:1, :1])
            nc.vector.tensor_copy(out=zT[:, k2:k2 + 1], in_=tzv[:, 0:1])
        for n in range(20):
            wdc = ixp.tile([P, 2 * TCONV], bf16, tag="wdc", bufs=4)
            nc.gpsimd.dma_start(
                out=wdc[:].rearrange("p (k c) -> p k c", c=TCONV),
                in_=wdict["Wdec"][:, n * 512:(n + 1) * 512]
                    .rearrange("(k p) c -> p k c", p=P))
            hp = pst.tile([P, TCONV], f32, tag="tpm", bufs=1)
            for k2 in range(2):
                nc.tensor.matmul(out=hp[:1, :TCONV], lhsT=zT[:, k2:k2 + 1],
                                 rhs=wdc[:, k2 * TCONV:(k2 + 1) * TCONV],
                                 start=(k2 == 0), stop=(k2 == 1))
            bdc = ixp.tile([1, TCONV], f32, tag="bdc", bufs=1)
            nc.gpsimd.dma_start(out=bdc[:], in_=wdict["bdec"][None, n * 512:(n + 1) * 512])
            hdo = stp.tile([1, TCONV], bf16, tag="hdo", bufs=2)
            nc.vector.tensor_tensor(out=hdo[:], in0=hp[:1, :TCONV], in1=bdc[:],
                                    op=OP.add)
            nc.sync.dma_start(
                out=tabs["thd"][n * 8:(n + 1) * 8, :64],
                in_=hdo[:].rearrange("o (v c) -> o v c", c=64))

        pool("pu3", tabs["thd"], False, 64, tabs["tu3"], False)
        conv("cd3", tabs["tu3"], False, 64, "Wd3", "bd3", tabs["td3"], False, V[3])
        pool("pu2", tabs["td3"], False, 64, tabs["tu2"], False)
        conv("cd2", tabs["tu2"], False, 64, "Wd2", "bd2", tabs["td2"], False, V[2])
        pool("pu1", tabs["td2"], False, 32, tabs["tu1"], False)
        conv("cd1", tabs["tu1"], False, 32, "Wd1", "bd1", tabs["td1"], False, V[1])
        pool("pu0", tabs["td1"], False, 32, tabs["tu0"], True)
        tc.strict_bb_all_engine_barrier()  # tu0 rotated
        conv("cd0", tabs["tu0"], True, 32, "Wd0", "bd0", tabs["ty"], True, V[0],
             yfuse=True)
        tc.strict_bb_all_engine_barrier()  # ty rotated
        # final conv: ty holds y_j = ELU(h_d0) @ Wf_j per vertex (4-ch slots);
        # out[v] = sum_j y_j[sp0[v, j]] + bf via 8B-elem gathers + tree add
        bfr = wp.tile([P, 4], f32, tag="bfr")
        nc.sync.dma_start(out=bfr[:], in_=wdict["bf_rep"][:])
        tyt = tabs["ty"]
        for gi2 in range(20):
            v0 = gi2 * 2048
            ixf = ixp.tile([P, 9 * 129], i16, tag="cidx", bufs=3, name="ixf")
            nc.gpsimd.dma_start(out=ixf[:],
                                in_=cfy_h[:, gi2 * 1161:(gi2 + 1) * 1161])
            gy = ixp.tile([P, 9 * 68], bf16, tag="gy", bufs=4, name="gy")
            for j in range(L):
                gv = (gy[:, j * 68:(j + 1) * 68]
                      .rearrange("p (q c) -> p q c", c=4)[:, :17, :])
                dma_gather(gv, tyt[ROT:, j * 4:(j + 1) * 4],
                           ixf[:, j * 129:(j + 1) * 129], 2048 + 16, 4, EP)
            gyv = (gy[:].rearrange("p (j x) -> p j x", x=68)[:, :, :64]
                   .rearrange("p j (q c) -> p j q c", c=4))
            nc.vector.tensor_tensor(out=gyv[:, 0:4], in0=gyv[:, 0:4],
                                    in1=gyv[:, 4:8], op=OP.add)
            nc.vector.tensor_tensor(out=gyv[:, 0:2], in0=gyv[:, 0:2],
                                    in1=gyv[:, 2:4], op=OP.add)
            nc.vector.tensor_tensor(out=gyv[:, 0:1], in0=gyv[:, 0:1],
                                    in1=gyv[:, 1:2], op=OP.add)
            nc.vector.tensor_tensor(out=gyv[:, 0:1], in0=gyv[:, 0:1],
                                    in1=gyv[:, 8:9], op=OP.add)
            fout = stp.tile([P, 64], f32, tag="fout", bufs=2)
            nc.vector.tensor_tensor(
                out=fout[:].rearrange("p (q c) -> p q c", c=4),
                in0=gyv[:, 0, :, :],
                in1=bfr[:, None, :].to_broadcast([P, 16, 4]), op=OP.add)
            nv = min(V[0] - v0, 2048)
            full_q = nv // P
            if full_q:
                nc.sync.dma_start(
                    out=out_h[v0: v0 + full_q * P, :]
                        .rearrange("(q p) c -> p q c", p=P),
                    in_=fout[:].rearrange("p (q c) -> p q c", c=4)[:, :full_q, :3])
            rem = nv - full_q * P
            if rem:
                nc.sync.dma_start(
                    out=out_h[v0 + full_q * P: v0 + nv, :],
                    in_=fout[:rem, full_q * 4: full_q * 4 + 3])
        es.close()

    if not nc.is_finalized():
        nc.finalize()
    return nc, ext


def _weights_dict(d, pr):
    w = {}
    for l in range(4):
        for nm in (f"We{l}", f"be{l}", f"Wd{l}", f"bd{l}"):
            w[nm] = np.asarray(d[nm], np.float32)
    for nm in ("Wf", "bf", "benc", "Wdec", "bdec"):
        w[nm] = np.asarray(d[nm], np.float32)
    w["Wenc_p"] = np.asarray(pr["Wenc_p"], np.float32)
    bf = np.asarray(d["bf"], np.float32)
    w["bf_rep"] = np.concatenate(
        [np.tile(bf[None, :], (128, 1)), np.zeros((128, 1), np.float32)], 1)
    wf9 = np.zeros((32, 9, 4), np.float32)
    wf9[:, :, :3] = np.asarray(d["Wf"], np.float32).reshape(9, 32, 3).transpose(1, 0, 2)
    w["Wf9"] = np.ascontiguousarray(wf9.reshape(32, 36))
    return w


def _run(inputs, trace=False, **kw):
    import ml_dtypes
    d = {k: np.asarray(v) for k, v in inputs.items()}
    pr = preprocess(d)
    nc, ext = build(pr, _weights_dict(d, pr))
    from concourse.bass_utils import run_bass_kernel_spmd
    gi = pr["xg_gi"]  # [nt0*512, 9]
    in_maps = []
    for i in range(B):
        m = dict(ext)
        xb = np.asarray(d["x"][i], np.float32)
        xg = xb[gi].reshape(-1, 27)  # [nt0*512, 27]
        m["xgT"] = np.ascontiguousarray(xg.T).astype(ml_dtypes.bfloat16)
        in_maps.append(m)
    res = run_bass_kernel_spmd(nc, in_maps, core_ids=list(range(B)),
                               trace=trace, **kw)
    out = np.stack([np.asarray(r["out"], np.float32) for r in res.results], 0)
    return out, res


def kernel(**inputs):
    return _run(inputs)[0]


# revision 27
# speedup vs baseline: 1.2016x; 1.0013x over previous
"""Trainium2 Bass kernel for the spiral-conv mesh autoencoder (nn_AE_45810121179173).

Data-parallel over batch: core i runs the full network for batch element i.
- gathers via InstDMAGatherAnt (SWDGE dma_gather) round-robined over 4 SWDGE
  queues; per-queue throughput ~8ns/idx (64B elems) so saturation needs 4
  concurrent gather instructions -> small uniform chunks + deep buffering.
- tables stored at 256B pitch ([*, 128] bf16) as dma_gather requires the
  source pitch to be a multiple of 256B; elem_size = real channel count.
- indices are int16; 40000-row tables use a rotated 65536-row buffer (row v
  stored at (v+32768)%65536, gather base view at +32768) so uint16-wrapped
  indices address the full range via signed-offset wraparound.
- first conv's x-gather is done host-side and laid out feature-major, so
  conv ce0 needs no gather and no PE transposes.
- pools: host-side degree-sorted padded-CSR gather; chunks group equal-width
  tiles so the weighted tree-sum is one DVE op per halving level per chunk.
- convs: gather (vertex-major) -> PE transpose to feature-major (PSUM f32,
  two tiles per bank-pair) -> scalar-engine copy to SBUF bf16 -> PSUM-
  accumulated matmuls -> pair-wide bias + ELU epilogue (scalar+DVE bf16)
  -> PE transpose back -> single table write per group.
Self-contained: hardcodes shapes; needs numpy + concourse runtime only.
"""
import sys

for _p in ("/opt/trn_rl_repo", "/root/.axon_site/_ro/trn_rl_repo"):
    if _p not in sys.path:
        sys.path.insert(0, _p)

import numpy as np

V = [40000, 10000, 2500, 625, 160]
L = 9
LAT = 256
B = 8
P = 128
TCONV = 512
G = 4
EP = 128          # table pitch in bf16 elements (256B)
ROT = 32768       # rotation for 40000-row tables
VB = 65536        # rotated table rows
NQ = 4            # SWDGE queues
POOL_SUB = 32     # target gather columns per pool chunk (32*128=4096 idx)
POOL_WMAX = 32    # a single tile may be up to this wide


def _pow2_up(x):
    n = 1
    while n < x:
        n *= 2
    return n


def _tpad(v, m):
    return ((v + m - 1) // m) * m


def _wrap16(flat):
    """[n] values -> [128, n//16+1] int16 FW layout; appends a positive
    sentinel column so the FW's trailing-negative-pad scan never drops real
    entries whose uint16 wrap is negative."""
    n = len(flat)
    assert n % 16 == 0
    u = np.asarray(flat).astype(np.uint16).astype(np.int16)
    u = np.concatenate([u, np.zeros(16, np.int16)])
    w = u.reshape(n // 16 + 1, 16).T  # [16, cols]; entry i at [i%16, i//16]
    return np.tile(w, (8, 1))


def _prep_pool(row, col, val, v_out):
    nnz = len(row)
    deg = np.bincount(row, minlength=v_out)
    perm = np.argsort(-deg, kind="stable").astype(np.int64)
    inv = np.empty(v_out, np.int64)
    inv[perm] = np.arange(v_out)
    order = np.argsort(row, kind="stable")
    col_s, val_s = col[order].astype(np.int64), np.asarray(val)[order].astype(np.float32)
    starts = np.zeros(v_out + 1, np.int64)
    np.cumsum(deg, out=starts[1:])
    ntiles = (v_out + P - 1) // P
    widths, idx_l, val_l = [], [], []
    for t in range(ntiles):
        dests = perm[t * P:(t + 1) * P]
        k = deg[dests]
        W = _pow2_up(max(1, int(k.max()) if len(k) else 1))
        assert W <= POOL_WMAX, f"pool tile width {W} exceeds {POOL_WMAX}"
        npd = len(dests)
        mask = np.arange(W)[None, :] < k[:, None]
        flat = np.minimum(starts[dests][:, None] + np.arange(W)[None, :], max(nnz - 1, 0))
        I = np.zeros((P, W), np.int64)
        A = np.zeros((P, W), np.float32)
        I[:npd] = np.where(mask, col_s[flat], 0)
        A[:npd] = np.where(mask, val_s[flat], 0.0)
        widths.append(W)
        idx_l.append(I)
        val_l.append(A)
    return dict(perm=perm, inv=inv, widths=widths,
                idx=np.concatenate(idx_l, 1),
                val=np.concatenate(val_l, 1).astype(np.float32), ntiles=ntiles)


def _pool_chunks(widths, brk=None):
    """Uniform-W chunking: consecutive equal-width tiles, wsum <= POOL_SUB,
    break at tile brk. A single tile wider than POOL_SUB gets its own chunk."""
    chunks = []
    t0 = 0
    ntiles = len(widths)
    while t0 < ntiles:
        W = widths[t0]
        t1, wsum = t0, 0
        while (t1 < ntiles and widths[t1] == W and wsum + W <= POOL_SUB
               and not (brk is not None and t1 == brk and t1 > t0)):
            wsum += W
            t1 += 1
        if t1 == t0:  # W > POOL_SUB
            t1, wsum = t0 + 1, W
        chunks.append((t0, t1, wsum))
        t0 = t1
    return chunks


def _pool_idx16(pp, brk=None):
    """Build per-chunk wrapped idx + chunk list; idx flat order i=col*128+p."""
    widths = pp["widths"]
    chunks = _pool_chunks(widths, brk)
    offs = np.concatenate([[0], np.cumsum(widths)]).astype(int)
    blocks = []
    for (t0, t1, wsum) in chunks:
        cols = pp["idx"][:, offs[t0]:offs[t1]]  # [128, wsum]
        blocks.append(_wrap16(cols.T.reshape(-1)))  # [128, wsum*8+1]
    return chunks, np.concatenate(blocks, 1).astype(np.int16), offs


def _conv_idx16(sp, inv):
    """[128, ntiles*290] int16: per-tile TWO wrap half-blocks (gr01 | gr23),
    each [128, 145]; within a half i=(grh*9+j)*128+p."""
    Vq = sp.shape[0]
    spr = (inv[sp] if inv is not None else np.asarray(sp).astype(np.int64))
    ntiles = (Vq + TCONV - 1) // TCONV
    pad = ntiles * TCONV - Vq
    if pad:
        spr = np.concatenate([spr, np.zeros((pad, L), np.int64)], 0)
    arr = spr.reshape(ntiles, G, P, L).transpose(0, 1, 3, 2)  # [t, gr, j, p]
    blocks = []
    for t in range(ntiles):
        blocks.append(_wrap16(arr[t, :2].reshape(18 * P)))  # [128, 145]
        blocks.append(_wrap16(arr[t, 2:].reshape(18 * P)))
    return np.concatenate(blocks, 1).astype(np.int16), ntiles


def preprocess(d):
    pr = {}
    for l in range(4):
        pr[f"pd{l}"] = _prep_pool(np.asarray(d[f"dr{l}"]), np.asarray(d[f"dc{l}"]),
                                  np.asarray(d[f"dv{l}"]), V[l + 1])
        pr[f"pu{l}"] = _prep_pool(np.asarray(d[f"ur{l}"]), np.asarray(d[f"uc{l}"]),
                                  np.asarray(d[f"uv{l}"]), V[l])
    sps = [np.asarray(d[f"sp{l}"]) for l in range(4)]
    # conv gather indices (int16-wrapped, per-tile blocks)
    for l in range(1, 4):
        pr[f"ce{l}"] = _conv_idx16(sps[l], pr[f"pd{l-1}"]["inv"])
    for l in range(4):
        pr[f"cd{l}"] = _conv_idx16(sps[l], pr[f"pu{l}"]["inv"])
    pr["cf"] = _conv_idx16(sps[0], None)
    # pool gather indices
    for pname in ["pd0", "pd1", "pd2", "pd3", "pu3", "pu2", "pu1", "pu0"]:
        pr[f"{pname}_g"] = _pool_idx16(pr[pname],
                                       brk=ROT // P if pname == "pu0" else None)
    # host x-gather layout for ce0: feature-major rhs [27, nt0*512]
    sp0 = sps[0].astype(np.int64)
    nt0 = (V[0] + TCONV - 1) // TCONV
    pad = nt0 * TCONV - V[0]
    sp0p = np.concatenate([sp0, np.zeros((pad, L), np.int64)], 0)
    pr["xg_gi"] = sp0p  # [nt0*512, 9]
    pr["nt0"] = nt0
    # final-conv y-gather idx: 20 groups x 2048 verts x 9 taps, wrap blocks
    sp0g = np.concatenate([sp0p, np.zeros((40960 - sp0p.shape[0], L), np.int64)], 0)
    blocks = []
    for gi2 in range(20):
        for j in range(L):
            blocks.append(_wrap16(sp0g[gi2 * 2048:(gi2 + 1) * 2048, j]))
    pr["cfy"] = np.concatenate(blocks, 1).astype(np.int16)
    Wenc = np.asarray(d["Wenc"]).reshape(V[4], 64, LAT)
    pr["Wenc_p"] = np.ascontiguousarray(
        Wenc[pr["pd3"]["perm"]].reshape(V[4] * 64, LAT))
    return pr


def build(pr, weights, dbg=False):
    from concourse import bacc, bass, tile, mybir
    from concourse.bass import exact_div
    import ml_dtypes

    f32, bf16, i32, i16 = (mybir.dt.float32, mybir.dt.bfloat16, mybir.dt.int32,
                           mybir.dt.int16)
    nc = bacc.Bacc(None, target_bir_lowering=False, num_swdge_queues=NQ)
    ext = {}
    qctr = [0]

    def qrr():
        q = qctr[0] % NQ
        qctr[0] += 1
        return q

    def dma_gather(out_ap, in_ap, idxs_ap, num_idxs, elem_size, elem_step):
        stride_bytes = elem_step * mybir.dt.size(in_ap.dtype)
        eng = nc.gpsimd
        return eng.add_instruction(
            mybir.InstDMAGatherAnt(
                name=eng.bass.get_next_instruction_name(),
                ins=[*eng.lower_ap_dma(in_ap, for_custom_bir_dma=True),
                     eng.lower_ap(idxs_ap),
                     eng.lower_val_access(eng.to_reg(num_idxs))],
                outs=[eng.lower_ap(out_ap)],
                transpose=False, num_idxs=num_idxs, elem_size=elem_size,
                stride_bytes_256=exact_div(stride_bytes, 256), gen_mode=0,
                single_packet=False, queue_num=qrr(),
                sbuf_tokens_per_rank=0, sbuf_free_dim_per_rank=0,
                sbuf_free_dim_pad_per_rank=0, sbuf_byte_offset=0))

    def ein(name, arr, dtype):
        h = nc.dram_tensor(name, list(arr.shape), dtype, kind="ExternalInput")
        if dtype == bf16:
            ext[name] = np.asarray(arr).astype(ml_dtypes.bfloat16)
        elif dtype == i32:
            ext[name] = np.asarray(arr).astype(np.int32)
        elif dtype == i16:
            ext[name] = np.asarray(arr).astype(np.int16)
        else:
            ext[name] = np.asarray(arr).astype(np.float32)
        return h

    nt0 = pr["nt0"]
    xg_h = nc.dram_tensor("xgT", [27, nt0 * TCONV], bf16, kind="ExternalInput")
    cfy_h = ein("cfy", pr["cfy"], i16)
    conv_idx = {}
    for cname in ["ce1", "ce2", "ce3", "cd3", "cd2", "cd1", "cd0", "cf"]:
        idx, nt = pr[cname]
        conv_idx[cname] = (ein(f"idx_{cname}", idx, i16), nt)
    pool_meta = {}
    for pname in ["pd0", "pd1", "pd2", "pd3", "pu3", "pu2", "pu1", "pu0"]:
        pp = pr[pname]
        chunks, idx16, offs = pr[f"{pname}_g"]
        pool_meta[pname] = dict(
            idx=ein(f"pidx_{pname}", idx16, i16),
            val=ein(f"pval_{pname}", pp["val"], f32),
            widths=pp["widths"], ntiles=pp["ntiles"], chunks=chunks, offs=offs)
    weights = dict(weights)
    weights["ident"] = np.eye(P, dtype=np.float32)
    weights["identb"] = np.eye(P, dtype=np.float32)
    wdict = {}
    for nm, arr in weights.items():
        wdict[nm] = ein(nm, arr, bf16 if (nm.startswith("W") or nm == "identb") else f32)

    # tables: [rows, 128] bf16 at 256B pitch (rot tables are 65536 rows)
    tabs = {}
    for nm, rows in dict(
            te0=VB, tp0=_tpad(V[1], P), te1=_tpad(V[1], TCONV),
            tp1=_tpad(V[2], P), te2=_tpad(V[2], TCONV), tp2=_tpad(V[3], P),
            te3=_tpad(V[3], TCONV), thd=V[4],
            tu3=_tpad(V[3], P), td3=_tpad(V[3], TCONV),
            tu2=_tpad(V[2], P), td2=_tpad(V[2], TCONV),
            tu1=_tpad(V[1], P), td1=_tpad(V[1], TCONV),
            tu0=VB, ty=VB).items():
        tabs[nm] = nc.dram_tensor(f"tab_{nm}", [rows, EP], bf16,
                                  kind="ExternalOutput" if dbg else "Internal")
    tp3_h = nc.dram_tensor("tab_tp3", [_tpad(V[4], P), 64], bf16,
                           kind="ExternalOutput" if dbg else "Internal")
    out_h = nc.dram_tensor("out", [V[0], 3], f32, kind="ExternalOutput")

    AT = mybir.ActivationFunctionType
    OP = mybir.AluOpType

    def src_view(tab, cin, rot):
        v3 = tab[:].rearrange("v (s c) -> v s c", c=cin)
        return v3[ROT:, 0, :] if rot else v3[:, 0, :]

    def rotpos(v0):
        return (v0 + ROT) % VB

    with tile.TileContext(nc) as tc:
        from contextlib import ExitStack
        es = ExitStack()
        wp = es.enter_context(tc.tile_pool(name="wp", bufs=1))
        gcv = es.enter_context(tc.tile_pool(name="gcv", bufs=8))
        gpl = es.enter_context(tc.tile_pool(name="gpl", bufs=5))
        gxp = es.enter_context(tc.tile_pool(name="gxp", bufs=2))
        ixp = es.enter_context(tc.tile_pool(name="ixp", bufs=4))
        rhp = es.enter_context(tc.tile_pool(name="rhp", bufs=5))
        elp = es.enter_context(tc.tile_pool(name="elp", bufs=2))
        pps = es.enter_context(tc.tile_pool(name="pps", bufs=2))
        stp = es.enter_context(tc.tile_pool(name="stp", bufs=3))
        pst = es.enter_context(tc.tile_pool(name="pst", bufs=2, space="PSUM"))
        pso = es.enter_context(tc.tile_pool(name="pso", bufs=1, space="PSUM"))

        idf = wp.tile([P, P], f32, tag="idf")
        nc.sync.dma_start(out=idf[:], in_=wdict["ident"][:])
        idb = wp.tile([P, P], bf16, tag="idb")
        nc.sync.dma_start(out=idb[:], in_=wdict["identb"][:])
        benc_sb = wp.tile([1, LAT], f32, tag="benc")
        nc.sync.dma_start(out=benc_sb[:], in_=wdict["benc"][None, :])

        wf9_sb = wp.tile([32, 36], bf16, tag="wf9")
        nc.sync.dma_start(out=wf9_sb[:], in_=wdict["Wf9"][:])
        wsb, bsb = {}, {}
        for nm in ["We0", "We1", "We2", "We3", "Wd0", "Wd1", "Wd2", "Wd3", "Wf"]:
            h = wdict[nm]
            rows, cout = h.shape
            nchunk = (rows + P - 1) // P
            t = wp.tile([min(P, rows), nchunk * cout], bf16, tag=f"w_{nm}")
            for ck in range(nchunk):
                r0, r1 = ck * P, min(rows, (ck + 1) * P)
                nc.sync.dma_start(out=t[: r1 - r0, ck * cout:(ck + 1) * cout],
                                  in_=h[r0:r1, :])
            wsb[nm] = (t, rows, cout)
        for nm in ["be0", "be1", "be2", "be3", "bd0", "bd1", "bd2", "bd3", "bf"]:
            h = wdict[nm]
            t = wp.tile([h.shape[0], 1], f32, tag=f"b_{nm}")
            nc.sync.dma_start(out=t[:], in_=h[:, None])
            bsb[nm] = t

        def conv(cname, src_tab, src_rot, cin, wname, bname, dst_tab, dst_rot,
                 Vq, elu=True, final=False, yfuse=False):
            YC = 36
            dense = cname == "ce0"
            if dense:
                ntiles = nt0
            else:
                idx_h, ntiles = conv_idx[cname]
            wt, wrows, cout = wsb[wname]
            bias = bsb[bname]
            spj = max(1, P // cin)
            nchunks = (L + spj - 1) // spj
            chunks = [(b * spj, min(L, (b + 1) * spj)) for b in range(nchunks)]
            NG = 4
            o_dt = f32 if final else bf16
            sview = None if dense else src_view(src_tab, cin, src_rot)
            ix_pre = {}

            def load_ix(c0p):
                if dense or c0p >= ntiles:
                    return
                ngp = min(NG, ntiles - c0p)
                ixn = ixp.tile([P, NG * 290], i16, tag="cidx", bufs=3,
                               name="ixn")
                nc.gpsimd.dma_start(out=ixn[:, : ngp * 290],
                                    in_=idx_h[:, c0p * 290:(c0p + ngp) * 290])
                ix_pre[c0p] = ixn

            load_ix(0)
            load_ix(NG)
            for c0 in range(0, ntiles, NG):
                ng = min(NG, ntiles - c0)
                npair = (ng + 1) // 2
                # --- inputs ---
                if dense:
                    gx = gxp.tile([27, NG * TCONV], bf16, tag="cgxT")
                    nc.gpsimd.dma_start(
                        out=gx[:, : ng * TCONV],
                        in_=xg_h[:, c0 * TCONV:(c0 + ng) * TCONV])
                    rhs = {0: gx}
                else:
                    ix = ix_pre.pop(c0)
                    gts = []
                    for t in range(ng):
                        g = gcv.tile([P, 38 * 64], bf16, tag="cg")
                        for hh in range(2):
                            gv = (g[:, hh * 19 * cin: (hh * 19 + 19) * cin]
                                  .rearrange("p (q c) -> p q c", c=cin))
                            dma_gather(gv, sview,
                                       ix[:, t * 290 + hh * 145:
                                          t * 290 + (hh + 1) * 145],
                                       18 * P + 16, cin, EP)
                        gts.append(g)
                    load_ix(c0 + 2 * NG)
                    # phase T: transposes in tile-pairs (bf16 PSUM), DVE copy out
                    rhs = {}
                    for bI, (j0, j1) in enumerate(chunks):
                        cb = (j1 - j0) * cin
                        r = rhp.tile([P, NG * TCONV], bf16, tag="rhs")
                        for h in range(npair):
                            th = min(2, ng - 2 * h)
                            tp = pst.tile([P, 2 * TCONV], bf16, tag="tpb")
                            for ti in range(th):
                                gt = gts[2 * h + ti]
                                for gr in range(G):
                                    sl = (gr * L + j0 if gr < 2
                                          else 19 + (gr - 2) * L + j0)
                                    nc.tensor.transpose(
                                        out=tp[:cb, (ti * G + gr) * P:(ti * G + gr + 1) * P],
                                        in_=gt[:, sl * cin:(sl + j1 - j0) * cin],
                                        identity=idb[:, :])
                            nc.vector.tensor_copy(
                                out=r[:cb, 2 * h * TCONV:(2 * h + th) * TCONV],
                                in_=tp[:cb, : th * TCONV])
                        rhs[bI] = r
                # phase M: one 4-bank PSUM tile for the whole group
                pos = pso.tile([64, NG * TCONV], f32, tag="po")
                for bI, (j0, j1) in enumerate(chunks):
                    cb = (j1 - j0) * cin
                    rsrc = rhs[0 if dense else bI]
                    for t in range(ng):
                        nc.tensor.matmul(
                            out=pos[:cout, t * TCONV:(t + 1) * TCONV],
                            lhsT=wt[:cb, bI * cout:(bI + 1) * cout],
                            rhs=rsrc[:cb, t * TCONV:(t + 1) * TCONV],
                            start=(bI == 0), stop=(bI == nchunks - 1))
                # phase E+B per pair: epilogue then back transposes
                if final:
                    tb = pst.tile([P, TCONV], f32, tag="tpm", bufs=1, name="tbf")
                else:
                    tb = pst.tile([P, 2 * TCONV], bf16, tag="tb", bufs=1, name="tbb")
                for h in range(npair):
                    th = min(2, ng - 2 * h)
                    nv5 = th * TCONV
                    pslc = pos[:cout, 2 * h * TCONV: 2 * h * TCONV + nv5]
                    if elu:
                        a1 = elp.tile([64, 2 * TCONV], f32, tag="a1")
                        nc.scalar.activation(out=a1[:cout, :nv5], in_=pslc,
                                             func=AT.Identity, bias=bias[:, :],
                                             scale=1.0)
                        m = elp.tile([64, 2 * TCONV], f32, tag="m")
                        nc.vector.tensor_scalar_min(out=m[:cout, :nv5],
                                                    in0=a1[:cout, :nv5], scalar1=0.0)
                        e = elp.tile([64, 2 * TCONV], f32, tag="e")
                        nc.scalar.activation(out=e[:cout, :nv5], in_=m[:cout, :nv5],
                                             func=AT.Exp)
                        src_o = elp.tile([64, 2 * TCONV], bf16, tag="ofin")
                        nc.vector.scalar_tensor_tensor(
                            out=src_o[:cout, :nv5], in0=e[:cout, :nv5], scalar=-1.0,
                            in1=a1[:cout, :nv5], op0=OP.add, op1=OP.max)
                    else:
                        src_o = elp.tile([64, 2 * TCONV], f32, tag="a1f")
                        nc.scalar.activation(out=src_o[:cout, :nv5], in_=pslc,
                                             func=AT.Identity, bias=bias[:, :],
                                             scale=1.0)
                    for ti in range(th):
                        t = 2 * h + ti
                        if yfuse:
                            yps = pst.tile([P, TCONV], f32, tag="tpm", bufs=1,
                                           name="yps")
                            nc.tensor.matmul(
                                out=yps[:YC, :],
                                lhsT=wf9_sb[:32, :YC],
                                rhs=src_o[:cout, ti * TCONV:(ti + 1) * TCONV],
                                start=True, stop=True)
                            ysb = elp.tile([40, TCONV], bf16, tag="ysb", bufs=2,
                                           name="ysb")
                            nc.scalar.copy(out=ysb[:YC, :], in_=yps[:YC, :])
                            for gr in range(G):
                                nc.tensor.transpose(
                                    out=tb[:, (t * G + gr) * YC:(t * G + gr + 1) * YC],
                                    in_=ysb[:YC, gr * P:(gr + 1) * P],
                                    identity=idb[:YC, :YC])
                            continue
                        for gr in range(G):
                            nc.tensor.transpose(
                                out=tb[:, (t * G + gr) * cout:(t * G + gr + 1) * cout],
                                in_=src_o[:cout, ti * TCONV + gr * P:
                                          ti * TCONV + (gr + 1) * P],
                                identity=(idf if final else idb)[:cout, :cout])
                cw = YC if yfuse else cout
                ost = stp.tile([P, NG * G * 64], o_dt,
                               tag=("costf" if final else "cost"),
                               bufs=(2 if final else 3))
                nc.vector.tensor_copy(out=ost[:, : ng * G * cw],
                                      in_=tb[:, : ng * G * cw])
                # write group (contiguous rows; split if the rotation wraps)
                v0 = c0 * TCONV
                if not final:
                    nrows = ng * TCONV
                    p0 = rotpos(v0) if dst_rot else v0
                    segs = ([(p0, 0, nrows)] if not dst_rot or p0 + nrows <= VB
                            else [(p0, 0, VB - p0), (0, VB - p0, nrows - (VB - p0))])
                    for (d0, s0, nr) in segs:
                        nc.sync.dma_start(
                            out=dst_tab[d0: d0 + nr, :cw]
                                .rearrange("(q p) c -> p q c", p=P),
                            in_=ost[:, s0 // P * cw: (s0 + nr) // P * cw]
                                .rearrange("p (q c) -> p q c", c=cw))
                else:
                    nv = min(Vq - v0, ng * TCONV)
                    full_q = nv // P
                    if full_q:
                        nc.sync.dma_start(
                            out=dst_tab[v0: v0 + full_q * P, :]
                                .rearrange("(q p) c -> p q c", p=P),
                            in_=ost[:, : full_q * cout]
                                .rearrange("p (q c) -> p q c", c=cout))
                    rem = nv - full_q * P
                    if rem:
                        nc.sync.dma_start(
                            out=dst_tab[v0 + full_q * P: v0 + nv, :],
                            in_=ost[:rem, full_q * cout:(full_q + 1) * cout])

        def pool(pname, src_tab, src_rot, C, dst_tab, dst_rot):
            meta = pool_meta[pname]
            idx_h, val_h = meta["idx"], meta["val"]
            chunks, offs = meta["chunks"], meta["offs"]
            sview = src_view(src_tab, C, src_rot)
            icol = 0
            for (t0, t1, wsum) in chunks:
                nt = t1 - t0
                W = wsum // nt
                ix = ixp.tile([P, POOL_WMAX * 8 + 1], i16, tag="pidx")
                vl = ixp.tile([P, POOL_WMAX], f32, tag="pval")
                nc.gpsimd.dma_start(out=ix[:, : wsum * 8 + 1],
                                    in_=idx_h[:, icol:icol + wsum * 8 + 1])
                nc.gpsimd.dma_start(out=vl[:, :wsum], in_=val_h[:, offs[t0]:offs[t1]])
                g = gpl.tile([P, (POOL_WMAX + 1) * 64], bf16, tag="pg")
                gv = g[:].rearrange("p (q c) -> p q c", c=C)[:, :wsum + 1, :]
                dma_gather(gv, sview, ix[:, : wsum * 8 + 1], wsum * P + 16, C, EP)
                s = pps.tile([P, POOL_WMAX * 64], f32, tag="ps")
                sv = s[:, : wsum * C].rearrange("p (t w c) -> p t w c", w=W, c=C)
                nc.vector.tensor_tensor(
                    out=sv,
                    in0=g[:, : wsum * C].rearrange("p (t w c) -> p t w c", w=W, c=C),
                    in1=vl[:, :wsum].rearrange("p (t w) -> p t w", w=W)[:, :, :, None]
                        .to_broadcast([P, nt, W, C]),
                    op=OP.mult)
                h = W
                while h > 1:
                    h //= 2
                    nc.vector.tensor_tensor(
                        out=sv[:, :, :h, :], in0=sv[:, :, :h, :],
                        in1=sv[:, :, h:2 * h, :], op=OP.add)
                ost = stp.tile([P, POOL_SUB * 64], bf16, tag="post")
                nc.vector.tensor_copy(out=ost[:, : nt * C], in_=sv[:, :, 0, :])
                p0 = rotpos(t0 * P) if dst_rot else t0 * P
                nc.sync.dma_start(
                    out=dst_tab[p0: p0 + nt * P, :C].rearrange("(q p) c -> p q c", p=P),
                    in_=ost[:, : nt * C].rearrange("p (q c) -> p q c", c=C))
                icol += wsum * 8 + 1

        # ---------------- network ----------------
        conv("ce0", None, False, 3, "We0", "be0", tabs["te0"], True, V[0])
        tc.strict_bb_all_engine_barrier()  # te0 rotated: gather range undeclared
        pool("pd0", tabs["te0"], True, 32, tabs["tp0"], False)
        conv("ce1", tabs["tp0"], False, 32, "We1", "be1", tabs["te1"], False, V[1])
        pool("pd1", tabs["te1"], False, 32, tabs["tp1"], False)
        conv("ce2", tabs["tp1"], False, 32, "We2", "be2", tabs["te2"], False, V[2])
        pool("pd2", tabs["te2"], False, 32, tabs["tp2"], False)
        conv("ce3", tabs["tp2"], False, 32, "We3", "be3", tabs["te3"], False, V[3])
        pool("pd3", tabs["te3"], False, 64, tp3_h, False)

        h4a = wp.tile([P, 64], bf16, tag="h4a")
        h4b = wp.tile([P, 64], bf16, tag="h4b")
        nc.sync.dma_start(out=h4a[:], in_=tp3_h[0:P, :])
        nc.sync.dma_start(out=h4b[:32, :], in_=tp3_h[P:160, :])
        h4T = wp.tile([64, 160], bf16, tag="h4T")
        tpa = pst.tile([P, TCONV], f32, tag="tpm", bufs=1)
        tpav = tpa[:].bitcast(bf16)
        nc.tensor.transpose(out=tpav[:64, 0:P], in_=h4a[:, :], identity=idb[:, :])
        nc.vector.tensor_copy(out=h4T[:, 0:P], in_=tpav[:64, 0:P])
        tpb = pst.tile([P, TCONV], f32, tag="tpm", bufs=1)
        tpbv = tpb[:].bitcast(bf16)
        nc.tensor.transpose(out=tpbv[:64, :32], in_=h4b[:32, :], identity=idb[:32, :32])
        nc.vector.tensor_copy(out=h4T[:, P:160], in_=tpbv[:64, :32])
        fl = wp.tile([P, 80], bf16, tag="fl")
        nc.vector.tensor_copy(out=fl[0:64, :], in_=h4T[:, 0:160:2])
        nc.vector.tensor_copy(out=fl[64:P, :], in_=h4T[:, 1:160:2])
        zps = pst.tile([P, TCONV], f32, tag="tpm", bufs=1)
        wev = wdict["Wenc_p"][:].rearrange("(k p) n -> p k n", p=P)
        for kc in range(8):
            wch = ixp.tile([P, 10 * LAT], bf16, tag="wenck", bufs=2)
            nc.gpsimd.dma_start(
                out=wch[:].rearrange("p (k n) -> p k n", n=LAT),
                in_=wev[:, kc * 10:(kc + 1) * 10, :])
            wv = wch[:].rearrange("p (k n) -> p k n", n=LAT)
            for k1 in range(10):
                k = kc * 10 + k1
                nc.tensor.matmul(out=zps[:1, :LAT], lhsT=fl[:, k:k + 1],
                                 rhs=wv[:, k1, :],
                                 start=(k == 0), stop=(k == 79))
        z_sb = wp.tile([1, LAT], bf16, tag="z_sb")
        nc.vector.tensor_tensor(out=z_sb[:], in0=zps[:1, :LAT], in1=benc_sb[:],
                                op=OP.add)
        zT = wp.tile([P, 2], bf16, tag="zT")
        for k2 in range(2):
            tz = pst.tile([P, TCONV], f32, tag="tpm", bufs=1)
            tzv = tz[:].bitcast(bf16)
            nc.tensor.transpose(out=tzv[:, 0:1], in_=z_sb[:, k2 * P:(k2 + 1) * P],
                                identity=idb[:1, :1])
            nc.vector.tensor_copy(out=zT[:, k2:k2 + 1], in_=tzv[:, 0:1])
        for n in range(20):
            wdc = ixp.tile([P, 2 * TCONV], bf16, tag="wdc", bufs=4)
            nc.gpsimd.dma_start(
                out=wdc[:].rearrange("p (k c) -> p k c", c=TCONV),
                in_=wdict["Wdec"][:, n * 512:(n + 1) * 512]
                    .rearrange("(k p) c -> p k c", p=P))
            hp = pst.tile([P, TCONV], f32, tag="tpm", bufs=1)
            for k2 in range(2):
                nc.tensor.matmul(out=hp[:1, :TCONV], lhsT=zT[:, k2:k2 + 1],
                                 rhs=wdc[:, k2 * TCONV:(k2 + 1) * TCONV],
                                 start=(k2 == 0), stop=(k2 == 1))
            bdc = ixp.tile([1, TCONV], f32, tag="bdc", bufs=1)
            nc.gpsimd.dma_start(out=bdc[:], in_=wdict["bdec"][None, n * 512:(n + 1) * 512])
            hdo = stp.tile([1, TCONV], bf16, tag="hdo", bufs=2)
            nc.vector.tensor_tensor(out=hdo[:], in0=hp[:1, :TCONV], in1=bdc[:],
                                    op=OP.add)
            nc.sync.dma_start(
                out=tabs["thd"][n * 8:(n + 1) * 8, :64],
                in_=hdo[:].rearrange("o (v c) -> o v c", c=64))

        pool("pu3", tabs["thd"], False, 64, tabs["tu3"], False)
        conv("cd3", tabs["tu3"], False, 64, "Wd3", "bd3", tabs["td3"], False, V[3])
        pool("pu2", tabs["td3"], False, 64, tabs["tu2"], False)
        conv("cd2", tabs["tu2"], False, 64, "Wd2", "bd2", tabs["td2"], False, V[2])
        pool("pu1", tabs["td2"], False, 32, tabs["tu1"], False)
        conv("cd1", tabs["tu1"], False, 32, "Wd1", "bd1", tabs["td1"], False, V[1])
        pool("pu0", tabs["td1"], False, 32, tabs["tu0"], True)
        tc.strict_bb_all_engine_barrier()  # tu0 rotated
        conv("cd0", tabs["tu0"], True, 32, "Wd0", "bd0", tabs["ty"], True, V[0],
             yfuse=True)
        tc.strict_bb_all_engine_barrier()  # ty rotated
        # final conv: ty holds y_j = ELU(h_d0) @ Wf_j per vertex (4-ch slots);
        # out[v] = sum_j y_j[sp0[v, j]] + bf via 8B-elem gathers + tree add
        bfr = wp.tile([P, 4], f32, tag="bfr")
        nc.sync.dma_start(out=bfr[:], in_=wdict["bf_rep"][:])
        tyt = tabs["ty"]
        cfix = {}

        def load_cfix(g2):
            if g2 >= 20:
                return
            ixn2 = ixp.tile([P, 9 * 129], i16, tag="cidx", bufs=3, name="ixn2")
            nc.gpsimd.dma_start(out=ixn2[:],
                                in_=cfy_h[:, g2 * 1161:(g2 + 1) * 1161])
            cfix[g2] = ixn2

        load_cfix(0)
        load_cfix(1)
        for gi2 in range(20):
            v0 = gi2 * 2048
            ixf = cfix.pop(gi2)
            gy = ixp.tile([P, 9 * 68], bf16, tag="gy", bufs=6, name="gy")
            for j in range(L):
                gv = (gy[:, j * 68:(j + 1) * 68]
                      .rearrange("p (q c) -> p q c", c=4)[:, :17, :])
                dma_gather(gv, tyt[ROT:, j * 4:(j + 1) * 4],
                           ixf[:, j * 129:(j + 1) * 129], 2048 + 16, 4, EP)
            load_cfix(gi2 + 2)
            gyv = (gy[:].rearrange("p (j x) -> p j x", x=68)[:, :, :64]
                   .rearrange("p j (q c) -> p j q c", c=4))
            nc.vector.tensor_tensor(out=gyv[:, 0:4], in0=gyv[:, 0:4],
                                    in1=gyv[:, 4:8], op=OP.add)
            nc.vector.tensor_tensor(out=gyv[:, 0:2], in0=gyv[:, 0:2],
                                    in1=gyv[:, 2:4], op=OP.add)
            nc.vector.tensor_tensor(out=gyv[:, 0:1], in0=gyv[:, 0:1],
                                    in1=gyv[:, 1:2], op=OP.add)
            nc.vector.tensor_tensor(out=gyv[:, 0:1], in0=gyv[:, 0:1],
                                    in1=gyv[:, 8:9], op=OP.add)
            fout = stp.tile([P, 64], f32, tag="fout", bufs=2)
            nc.vector.tensor_tensor(
                out=fout[:].rearrange("p (q c) -> p q c", c=4),
                in0=gyv[:, 0, :, :],
                in1=bfr[:, None, :].to_broadcast([P, 16, 4]), op=OP.add)
            nv = min(V[0] - v0, 2048)
            full_q = nv // P
            if full_q:
                nc.sync.dma_start(
                    out=out_h[v0: v0 + full_q * P, :]
                        .rearrange("(q p) c -> p q c", p=P),
                    in_=fout[:].rearrange("p (q c) -> p q c", c=4)[:, :full_q, :3])
            rem = nv - full_q * P
            if rem:
                nc.sync.dma_start(
                    out=out_h[v0 + full_q * P: v0 + nv, :],
                    in_=fout[:rem, full_q * 4: full_q * 4 + 3])
        es.close()

    if not nc.is_finalized():
        nc.finalize()
    return nc, ext


def _weights_dict(d, pr):
    w = {}
    for l in range(4):
        for nm in (f"We{l}", f"be{l}", f"Wd{l}", f"bd{l}"):
            w[nm] = np.asarray(d[nm], np.float32)
    for nm in ("Wf", "bf", "benc", "Wdec", "bdec"):
        w[nm] = np.asarray(d[nm], np.float32)
    w["Wenc_p"] = np.asarray(pr["Wenc_p"], np.float32)
    bf = np.asarray(d["bf"], np.float32)
    w["bf_rep"] = np.concatenate(
        [np.tile(bf[None, :], (128, 1)), np.zeros((128, 1), np.float32)], 1)
    wf9 = np.zeros((32, 9, 4), np.float32)
    wf9[:, :, :3] = np.asarray(d["Wf"], np.float32).reshape(9, 32, 3).transpose(1, 0, 2)
    w["Wf9"] = np.ascontiguousarray(wf9.reshape(32, 36))
    return w


def _run(inputs, trace=False, **kw):
    import ml_dtypes
    d = {k: np.asarray(v) for k, v in inputs.items()}
    pr = preprocess(d)
    nc, ext = build(pr, _weights_dict(d, pr))
    from concourse.bass_utils import run_bass_kernel_spmd
    gi = pr["xg_gi"]  # [nt0*512, 9]
    in_maps = []
    for i in range(B):
        m = dict(ext)
        xb = np.asarray(d["x"][i], np.float32)
        xg = xb[gi].reshape(-1, 27)  # [nt0*512, 27]
        m["xgT"] = np.ascontiguousarray(xg.T).astype(ml_dtypes.bfloat16)
        in_maps.append(m)
    res = run_bass_kernel_spmd(nc, in_maps, core_ids=list(range(B)),
                               trace=trace, **kw)
    out = np.stack([np.asarray(r["out"], np.float32) for r in res.results], 0)
    return out, res


def kernel(**inputs):
    return _run(inputs)[0]
